# revision 1
# baseline (speedup 1.0000x reference)
"""Trainium2 Bass kernel for nn_BoundaryControlledMixer (4-layer Mamba stack +
boundary-controlled gate), tensor-parallel over d_inner across 8 NeuronCores.

Per core (owns E_loc = 192 of E = 1536 channels, full batch/sequence):
  - Activations flow feature-major [feat, token] so projections chain on the
    PE without transposes (matmul contracts the partition dim).
  - Selective scan: DVE tensor_tensor_scan (state = dA*state + b, fp32 state)
    over 24 row-tiles of the (n, e) grid: tile k holds rows r = n*8 + e_l,
    e = 8k + e_l, n = r//8, scanned along the full token axis [128, T].
    Batch reset via dA[:, L] = 0.  Inputs built by PE 0/1-replication
    (dt -> dtrep -> exp = dA) and log-doubling DMA (u = dt*xc -> ubrep),
    b = ubrep * Brep.  y = sum_n C*h via a 0/1 pooling matmul.
"""

import numpy as np

import concourse.bacc as bacc
import concourse.bass as bass
import concourse.mybir as mybir
import concourse.tile as tile
from concourse import masks
from concourse.bass_utils import run_bass_kernel_spmd

FP32 = mybir.dt.float32
BF16 = mybir.dt.bfloat16
AF = mybir.ActivationFunctionType
OP = mybir.AluOpType
AX = mybir.AxisListType

B, L, DM, NL = 2, 1024, 768, 4
E, N, K, R = 2 * DM, 16, 4, DM // 16
NC = 8
ELOC = E // NC            # 192
T = B * L                 # 2048
Q = 128
NCH = T // Q              # 16
EPS = 1e-5
DTILES = DM // 128        # 6
GDM = DM // NC            # 96
LPD = L + 2 * K           # padded per-batch xp row
NKT = N * ELOC // 128     # 24 scan tiles
RD = R + 1                # 49

_CACHE = {}
_DEBUG = False


def _etiles():
    return [(0, 128), (128, 64)]


def _build():
    nc = bacc.Bacc("TRN2", target_bir_lowering=False, debug=False)

    x_d = nc.dram_tensor("x", [T, DM], FP32, kind="ExternalInput")
    bprob_d = nc.dram_tensor("bprob", [1, T], BF16, kind="ExternalInput")
    w_in_d = nc.dram_tensor("w_in", [NL, 128, 6 * 2 * ELOC], BF16, kind="ExternalInput")
    conv_w_d = nc.dram_tensor("conv_w", [NL, 128, 2 * K], FP32, kind="ExternalInput")
    conv_b_d = nc.dram_tensor("conv_b", [NL, 128, 2], FP32, kind="ExternalInput")
    w_xp_d = nc.dram_tensor("w_xp", [NL, 128, 2 * (R + 2 * N)], BF16, kind="ExternalInput")
    w_dt_d = nc.dram_tensor("w_dt", [NL, R, ELOC], BF16, kind="ExternalInput")
    dtb_d = nc.dram_tensor("dtb", [NL, 128, 2], FP32, kind="ExternalInput")
    w_out_d = nc.dram_tensor("w_out", [NL, 128, 2 * DM], BF16, kind="ExternalInput")
    ln_d = nc.dram_tensor("lnp", [NL, 128, 12], FP32, kind="ExternalInput")
    ssmd_d = nc.dram_tensor("ssmd", [NL, 128, 2], FP32, kind="ExternalInput")
    w_c1_d = nc.dram_tensor("w_c1", [128, 7 * GDM], BF16, kind="ExternalInput")
    b_c1_d = nc.dram_tensor("b_c1", [GDM, 1], FP32, kind="ExternalInput")
    w_c2_d = nc.dram_tensor("w_c2", [GDM + 1, DM], BF16, kind="ExternalInput")
    nrm_d = nc.dram_tensor("nrm", [4, DM], FP32, kind="ExternalInput")
    nrmc_d = nc.dram_tensor("nrmc", [128, 12], FP32, kind="ExternalInput")
    repa_d = nc.dram_tensor("repa", [128, 16 * 128], BF16, kind="ExternalInput")
    repb_d = nc.dram_tensor("repb", [64, 8 * 128], BF16, kind="ExternalInput")
    pool_d = nc.dram_tensor("poolm", [128, NKT * 128], BF16, kind="ExternalInput")
    repn_d = nc.dram_tensor("repn", [16, 128], BF16, kind="ExternalInput")
    acol_d = nc.dram_tensor("acol", [128, 1], FP32, kind="ExternalInput")

    out_d = nc.dram_tensor("out", [T, DM], FP32, kind="ExternalOutput")
    gate_d = nc.dram_tensor("gate", [T, DM], FP32, kind="ExternalOutput")
    dbg = {}
    if _DEBUG:
        dbg["dt"] = nc.dram_tensor("dbg_dt", [ELOC, T], FP32, kind="ExternalOutput")
        dbg["yssm"] = nc.dram_tensor("dbg_yssm", [ELOC, T], FP32, kind="ExternalOutput")
        dbg["xc"] = nc.dram_tensor("dbg_xc", [ELOC, T], FP32, kind="ExternalOutput")
        dbg["hln"] = nc.dram_tensor("dbg_hln", [DM, T], FP32, kind="ExternalOutput")

    with tile.TileContext(nc) as tc:
        with tc.tile_pool(name="const", bufs=1) as constp, \
             tc.tile_pool(name="persist", bufs=1) as pers, \
             tc.tile_pool(name="wts", bufs=1) as wpool, \
             tc.tile_pool(name="act", bufs=1) as actp, \
             tc.tile_pool(name="st2", bufs=1) as st2, \
             tc.tile_pool(name="scn", bufs=1) as scn, \
             tc.tile_pool(name="ps_rep", bufs=1, space="PSUM") as ps_rep, \
             tc.tile_pool(name="ps_pool", bufs=1, space="PSUM") as ps_pool, \
             tc.tile_pool(name="ps_mm", bufs=2, space="PSUM") as ps_mm, \
             tc.tile_pool(name="dram", bufs=2, space="DRAM") as dramp:

            def pmm(shape, dt=FP32):
                return ps_mm.tile(shape, dt, name="pmm", tag="pmm")

            # ---------- constants ----------
            ident32 = constp.tile([128, 128], FP32)
            masks.make_identity(nc, ident32[:])
            ident16 = constp.tile([128, 128], BF16)
            masks.make_identity(nc, ident16[:])
            onesrow16 = constp.tile([1, 512], BF16)
            nc.gpsimd.memset(onesrow16[:], 1.0)
            halfcol32 = constp.tile([128, 1], FP32)
            nc.gpsimd.memset(halfcol32[:], 0.5)
            halfcol16 = constp.tile([128, 1], BF16)
            nc.gpsimd.memset(halfcol16[:], 0.5)
            eps_ap = constp.tile([128, 1], FP32)
            nc.gpsimd.memset(eps_ap[:], EPS)
            nrmc = constp.tile([128, 12], FP32)
            nc.sync.dma_start(nrmc[:], nrmc_d[:])
            repa = constp.tile([128, 16 * 128], BF16)
            nc.sync.dma_start(repa[:], repa_d[:])
            repb = constp.tile([64, 8 * 128], BF16)
            nc.sync.dma_start(repb[:], repb_d[:])
            poolm = constp.tile([128, NKT * 128], BF16)
            nc.sync.dma_start(poolm[:], pool_d[:])
            repn = constp.tile([16, 128], BF16)
            nc.sync.dma_start(repn[:], repn_d[:])
            acol = constp.tile([128, 1], FP32)
            nc.sync.dma_start(acol[:], acol_d[:])

            # ---------- x -> feature-major fp32 residual ----------
            residual = [pers.tile([128, T], FP32, name=f"res{j}") for j in range(DTILES)]
            for c in range(NCH):
                x_tm_c = st2.tile([128, DM], FP32, name="x_tm_c", tag="x_tm_c")
                nc.sync.dma_start(x_tm_c[:], x_d[c * Q:(c + 1) * Q, :])
                for j in range(DTILES):
                    pt = pmm([128, 128])
                    nc.tensor.transpose(pt[:], x_tm_c[:, j * 128:(j + 1) * 128], ident32[:])
                    nc.scalar.copy(residual[j][:, c * Q:(c + 1) * Q], pt[:])

            # ---------- fused feature-major LayerNorm ----------
            def ln_fm(lnw_aps, lnb_aps, consume, dbg_dst=None):
                stats16 = actp.tile([1, T], BF16, name="stats16", tag="stats16")
                stats16b = actp.tile([1, T], BF16, name="stats16b", tag="stats16b")

                for f in range(T // 512):
                    fs = slice(f * 512, (f + 1) * 512)
                    sp1 = pmm([1, 512])
                    sp2 = pmm([1, 512])
                    for j in range(DTILES):
                        nc.tensor.matmul(sp1[:], halfcol32, residual[j][:, fs],
                                         start=(j == 0), stop=(j == DTILES - 1))
                    nc.scalar.activation(stats16[0:1, fs], sp1[:], AF.Copy, scale=2.0 / DM)
                    for j in range(DTILES):
                        sqj = st2.tile([128, 512], BF16, name="ln_sqj", tag="ln_sqj")
                        nc.vector.tensor_tensor(sqj[:], residual[j][:, fs],
                                                residual[j][:, fs], OP.mult)
                        nc.tensor.matmul(sp2[:], halfcol16[:], sqj[:],
                                         start=(j == 0), stop=(j == DTILES - 1))
                    nc.scalar.activation(stats16b[0:1, fs], sp2[:], AF.Copy, scale=2.0 / DM)

                for f in range(T // 512):
                    fs = slice(f * 512, (f + 1) * 512)
                    rp = pmm([128, 512])
                    nc.tensor.matmul(rp[:], onesrow16[:1, :128], stats16[0:1, fs],
                                     start=True, stop=True)
                    meanr = st2.tile([128, 512], FP32, name="ln_meanr", tag="ln_meanr")
                    nc.scalar.copy(meanr[:], rp[:])
                    rp2 = pmm([128, 512])
                    nc.tensor.matmul(rp2[:], onesrow16[:1, :128], stats16b[0:1, fs],
                                     start=True, stop=True)
                    invr = st2.tile([128, 512], FP32, name="ln_invr", tag="ln_invr")
                    nc.scalar.copy(invr[:], rp2[:])
                    # var = E[x^2] - mean^2 ; inv = exp(-0.5*ln(var+eps))
                    c2r = st2.tile([128, 512], FP32, name="ln_c2r", tag="ln_c2r")
                    nc.vector.tensor_tensor(c2r[:], meanr[:], meanr[:], OP.mult)
                    nc.vector.tensor_tensor(invr[:], invr[:], c2r[:], OP.subtract)
                    nc.scalar.activation(invr[:], invr[:], AF.Ln, bias=eps_ap[:])
                    nc.scalar.activation(invr[:], invr[:], AF.Exp, scale=-0.5)
                    nc.vector.tensor_tensor(c2r[:], meanr[:], invr[:], OP.mult)
                    slices = []
                    for j in range(DTILES):
                        tmp = st2.tile([128, 512], BF16, name="ln_tmp", tag="ln_tmp", bufs=2)
                        nc.vector.tensor_tensor(tmp[:], residual[j][:, fs], invr[:], OP.mult)
                        nc.vector.tensor_tensor(tmp[:], tmp[:], c2r[:], OP.subtract)
                        hlnf = st2.tile([128, 512], BF16, name="hlnf", tag=f"hlnf{j}")
                        nc.scalar.activation(hlnf[:], tmp[:], AF.Identity,
                                             scale=lnw_aps[j], bias=lnb_aps[j])
                        slices.append(hlnf)
                        if dbg_dst is not None:
                            hld = st2.tile([128, 512], FP32, name="hld", tag="hld")
                            nc.vector.tensor_copy(hld[:], hlnf[:])
                            nc.sync.dma_start(dbg_dst[j * 128:(j + 1) * 128, fs], hld[:])
                    consume(f, slices)

            # ================= layers =================
            for li in range(NL):
                w_in = wpool.tile([128, 6 * 2 * ELOC], BF16, name="w_in_sb", tag="w_in_sb")
                nc.sync.dma_start(w_in[:], w_in_d[li])
                w_cw = wpool.tile([128, 2 * K], FP32, name="w_cw_sb", tag="w_cw_sb")
                nc.sync.dma_start(w_cw[:], conv_w_d[li])
                w_cb = wpool.tile([128, 2], FP32, name="w_cb_sb", tag="w_cb_sb")
                nc.sync.dma_start(w_cb[:], conv_b_d[li])
                w_xp = wpool.tile([128, 2 * (R + 2 * N)], BF16, name="w_xp_sb", tag="w_xp_sb")
                nc.sync.dma_start(w_xp[:], w_xp_d[li])
                w_dt = wpool.tile([R, ELOC], BF16, name="w_dt_sb", tag="w_dt_sb")
                nc.sync.dma_start(w_dt[:], w_dt_d[li])
                dtb = wpool.tile([128, 2], FP32, name="dtb_sb", tag="dtb_sb")
                nc.sync.dma_start(dtb[:], dtb_d[li])
                w_out = wpool.tile([128, 2 * DM], BF16, name="w_out_sb", tag="w_out_sb")
                nc.sync.dma_start(w_out[:], w_out_d[li])
                w_ln = wpool.tile([128, 12], FP32, name="w_ln_sb", tag="w_ln_sb")
                nc.sync.dma_start(w_ln[:], ln_d[li])
                w_D = wpool.tile([128, 2], FP32, name="w_D_sb", tag="w_D_sb")
                nc.sync.dma_start(w_D[:], ssmd_d[li])

                # ---- LN fused with in_proj ----
                xp_t = [actp.tile([128, B * LPD], BF16, name="xp_pad0", tag="xp_pad0"),
                        actp.tile([64, B * LPD], BF16, name="xp_pad1", tag="xp_pad1")]
                z_dram = dramp.tile([ELOC, T], BF16, name="z_dram", tag="z_dram")
                for ti in range(2):
                    nc.vector.memset(xp_t[ti][:, 0:K], 0.0)
                    nc.vector.memset(xp_t[ti][:, LPD:LPD + K], 0.0)

                def padcol(fs, fl):
                    b_ = fs // L
                    off = b_ * LPD + K + (fs - b_ * L)
                    return slice(off, off + fl)

                def consume_inproj(f, sl6):
                    fs = f * 512
                    for mt in range(3):
                        pt = pmm([128, 512])
                        for kt in range(DTILES):
                            nc.tensor.matmul(
                                pt[:], w_in[:, kt * 384 + mt * 128:kt * 384 + (mt + 1) * 128],
                                sl6[kt][:], start=(kt == 0), stop=(kt == DTILES - 1))
                        if mt == 0:
                            nc.scalar.copy(xp_t[0][:, padcol(fs, 512)], pt[:])
                        elif mt == 1:
                            nc.scalar.copy(xp_t[1][:, padcol(fs, 512)], pt[0:64, :])
                            zst = st2.tile([64, 512], BF16, name="zst", tag="opf", bufs=2)
                            nc.scalar.copy(zst[:], pt[64:128, :])
                            nc.sync.dma_start(z_dram[0:64, fs:fs + 512], zst[:])
                        else:
                            zst2 = st2.tile([64, 512], BF16, name="zst2", tag="opf", bufs=2)
                            nc.scalar.copy(zst2[:], pt[0:64, :])
                            nc.sync.dma_start(z_dram[64:128, fs:fs + 512], zst2[:])
                            zst3 = st2.tile([64, 512], BF16, name="zst3", tag="opf", bufs=2)
                            nc.scalar.copy(zst3[:], pt[64:128, :])
                            nc.sync.dma_start(z_dram[128:192, fs:fs + 512], zst3[:])

                ln_fm([w_ln[:, 2 * j:2 * j + 1] for j in range(DTILES)],
                      [w_ln[:, 2 * j + 1:2 * j + 2] for j in range(DTILES)],
                      consume_inproj,
                      dbg_dst=dbg["hln"] if (_DEBUG and li == 0) else None)

                # ---- conv + silu ----
                xc = [actp.tile([128, T], BF16, name="xc0", tag="xc0"),
                      actp.tile([64, T], BF16, name="xc1", tag="xc1")]
                for ti, (eo, el) in enumerate(_etiles()):
                    for b_ in range(B):
                        acc = st2.tile([el, L], FP32, name="cacc", tag="cacc", bufs=1)
                        cb = b_ * LPD + K
                        nc.vector.tensor_scalar(acc[:], xp_t[ti][:el, cb - 3:cb - 3 + L],
                                                w_cw[0:el, ti * K:ti * K + 1], None, OP.mult)
                        for j in range(1, K):
                            nc.vector.scalar_tensor_tensor(
                                acc[:], xp_t[ti][:el, cb - 3 + j:cb - 3 + j + L],
                                w_cw[0:el, ti * K + j:ti * K + j + 1],
                                acc[:], OP.mult, OP.add)
                        nc.scalar.activation(xc[ti][:el, b_ * L:(b_ + 1) * L], acc[:],
                                             AF.Silu, bias=w_cb[0:el, ti:ti + 1])
                if _DEBUG and li == 0:
                    for ti, (eo, el) in enumerate(_etiles()):
                        xcd = st2.tile([el, T], FP32, name="xcd", tag="xcd")
                        nc.vector.tensor_copy(xcd[:], xc[ti][:el, :])
                        nc.sync.dma_start(dbg["xc"][eo:eo + el, :], xcd[:])

                # ---- x_proj partial + AllReduce ----
                dbl_in = dramp.tile([R + 2 * N, T], FP32, name="dbl_in", tag="dbl_in")
                dbl_out = dramp.tile([R + 2 * N, T], FP32, name="dbl_out", tag="dbl_out")
                for f in range(T // 512):
                    fs = slice(f * 512, (f + 1) * 512)
                    pt = pmm([80, 512])
                    for ti, (eo, el) in enumerate(_etiles()):
                        nc.tensor.matmul(pt[:], w_xp[0:el, ti * 80:(ti + 1) * 80],
                                         xc[ti][:el, fs], start=(ti == 0), stop=(ti == 1))
                    dblf = st2.tile([80, 512], FP32, name="dblf", tag="dblf")
                    nc.scalar.copy(dblf[:], pt[:])
                    nc.sync.dma_start(dbl_in[:, fs], dblf[:])
                nc.gpsimd.collective_compute("AllReduce", OP.add,
                                             replica_groups=[list(range(NC))],
                                             ins=[dbl_in[:]], outs=[dbl_out[:]])

                # ---- dt path: softplus(w_dt @ dbl + b) e-major ----
                dtf16 = scn.tile([R, T], BF16, name="dtf16", tag="dtf16")
                for h_ in range(2):
                    hsl = slice(h_ * 1024, (h_ + 1) * 1024)
                    dtf32 = scn.tile([R, 1024], FP32, name="dtf32", tag="dstage")
                    nc.sync.dma_start(dtf32[:], dbl_out[0:R, hsl])
                    nc.vector.tensor_copy(dtf16[:, hsl], dtf32[:])

                dt16 = [scn.tile([128, T], BF16, name="dt16_0", tag="dt16_0"),
                        scn.tile([64, T], BF16, name="dt16_1", tag="dt16_1")]
                for ti, (eo, el) in enumerate(_etiles()):
                    for h_ in range(2):
                        hsl = slice(h_ * 1024, (h_ + 1) * 1024)
                        ptd = ps_rep.tile([128, 1024], FP32, name="ptd", tag="rep")
                        for cc in range(2):
                            csl = slice(h_ * 1024 + cc * 512, h_ * 1024 + (cc + 1) * 512)
                            psl = slice(cc * 512, (cc + 1) * 512)
                            nc.tensor.matmul(ptd[0:el, psl],
                                             w_dt[:, eo:eo + el], dtf16[:, csl],
                                             start=True, stop=True)
                        e1 = scn.tile([el, 1024], FP32, name="e1", tag="dstage")
                        nc.scalar.activation(e1[:], ptd[0:el, :], AF.Exp,
                                             bias=dtb[0:el, ti:ti + 1])
                        nc.scalar.activation(dt16[ti][:el, hsl], e1[:], AF.Ln, bias=1.0)
                if _DEBUG and li == 0:
                    dtd = st2.tile([128, T], FP32, name="dtd", tag="dtd")
                    nc.vector.tensor_copy(dtd[:], dt16[0][:])
                    nc.sync.dma_start(dbg["dt"][0:128, :], dtd[:])
                    dtd2 = st2.tile([64, T], FP32, name="dtd2", tag="dtd")
                    nc.vector.tensor_copy(dtd2[:], dt16[1][:])
                    nc.sync.dma_start(dbg["dt"][128:192, :], dtd2[:])

                # ---- u = dt * xc ; B/C replication ----
                u16 = [scn.tile([128, T], BF16, name="u16_0", tag="u16_0"),
                       scn.tile([64, T], BF16, name="u16_1", tag="u16_1")]
                for ti, (eo, el) in enumerate(_etiles()):
                    nc.vector.tensor_tensor(u16[ti][:el, :], dt16[ti][:el, :],
                                            xc[ti][:el, :], OP.mult)

                Brep = scn.tile([128, T], BF16, name="Brep", tag="Brep")
                Crep = scn.tile([128, T], BF16, name="Crep", tag="Crep")
                for si, dst in ((0, Brep), (1, Crep)):
                    src16 = scn.tile([16, T], BF16, name=f"bc16_{si}", tag="dtf16")
                    for h_ in range(2):
                        hsl = slice(h_ * 1024, (h_ + 1) * 1024)
                        s32 = scn.tile([16, 1024], FP32, name="s32", tag="dstage")
                        nc.sync.dma_start(s32[:], dbl_out[R + si * N:R + (si + 1) * N, hsl])
                        nc.vector.tensor_copy(src16[:, hsl], s32[:])
                    for h_ in range(2):
                        hsl = slice(h_ * 1024, (h_ + 1) * 1024)
                        prr = ps_rep.tile([128, 1024], FP32, name="prr", tag="rep")
                        for cc in range(2):
                            psl = slice(cc * 512, (cc + 1) * 512)
                            csl = slice(h_ * 1024 + cc * 512, h_ * 1024 + (cc + 1) * 512)
                            nc.tensor.matmul(prr[:, psl], repn[:], src16[:, csl],
                                             start=True, stop=True)
                        nc.scalar.copy(dst[:, hsl], prr[:])

                # ---- scan tiles ----
                y_fm = [actp.tile([128, T], BF16, name="yfm0", tag="yfm0"),
                        actp.tile([64, T], BF16, name="yfm1", tag="yfm1")]
                for g, (ks, el_in) in enumerate(((range(0, 16), 128),
                                                 (range(16, 24), 64))):
                    pp = ps_pool.tile([128, T], FP32, name="pp", tag="pp")
                    rows_out = 128 if g == 0 else 64
                    for k in ks:
                        kk = k - 16 * g
                        repm = repa if g == 0 else repb
                        for b_ in range(B):
                            bsl = slice(b_ * L, (b_ + 1) * L)
                            # dt replication -> dA = exp(a * dtrep)
                            prp = ps_rep.tile([128, 1024], FP32, name="prp", tag="rep")
                            for cc in range(2):
                                psl = slice(cc * 512, (cc + 1) * 512)
                                csl = slice(b_ * L + cc * 512, b_ * L + (cc + 1) * 512)
                                nc.tensor.matmul(prp[:, psl],
                                                 repm[:, kk * 128:(kk + 1) * 128],
                                                 dt16[g][:el_in, csl],
                                                 start=True, stop=True)
                            dA = scn.tile([128, L], FP32, name="dA", tag="dA", bufs=2)
                            nc.scalar.activation(dA[:], prp[:], AF.Exp,
                                                 scale=acol[:, 0:1])
                            # ubrep: 16 parallel row-group copies
                            ubrep = scn.tile([128, L], BF16, name="ubrep",
                                             tag="ubrep", bufs=2)
                            for j in range(16):
                                nc.sync.dma_start(ubrep[8 * j:8 * j + 8, :],
                                                  u16[g][8 * kk:8 * kk + 8, bsl])
                            bk = scn.tile([128, L], BF16, name="bk", tag="bk", bufs=2)
                            nc.vector.tensor_tensor(bk[:], ubrep[:], Brep[:, bsl],
                                                    OP.mult)
                            hk = scn.tile([128, L], BF16, name="hk", tag="ubrep", bufs=2)
                            nc.vector.tensor_tensor_scan(hk[:], dA[:], bk[:], 0.0,
                                                         OP.mult, OP.add)
                            yck = scn.tile([128, L], BF16, name="yck", tag="bk", bufs=2)
                            nc.vector.tensor_tensor(yck[:], hk[:], Crep[:, bsl],
                                                    OP.mult)
                            # pool: accumulate sum_n C*h into e-rows
                            for cc in range(2):
                                psl = slice(b_ * L + cc * 512, b_ * L + (cc + 1) * 512)
                                nc.tensor.matmul(pp[0:rows_out, psl],
                                                 poolm[:, k * 128:k * 128 + rows_out],
                                                 yck[:, cc * 512:(cc + 1) * 512],
                                                 start=(k == ks[0]), stop=(k == ks[-1]))
                    nc.vector.tensor_copy(y_fm[g][0:rows_out, :], pp[0:rows_out, :])
                if _DEBUG and li == 0:
                    for ti, (eo, el) in enumerate(_etiles()):
                        ydd = st2.tile([el, T], FP32, name="ydd", tag="xcd")
                        nc.vector.tensor_copy(ydd[:], y_fm[ti][:el, :])
                        nc.sync.dma_start(dbg["yssm"][eo:eo + el, :], ydd[:])

                # ---- D-term, z-gate ----
                zt = [scn.tile([128, T], BF16, name="zt0", tag="u16_0"),
                      scn.tile([64, T], BF16, name="zt1", tag="u16_1")]
                for ti, (eo, el) in enumerate(_etiles()):
                    nc.sync.dma_start(zt[ti][:el, :], z_dram[eo:eo + el, :])
                    nc.vector.scalar_tensor_tensor(y_fm[ti][:el, :], xc[ti][:el, :],
                                                   w_D[0:el, ti:ti + 1], y_fm[ti][:el, :],
                                                   OP.mult, OP.add)
                    nc.scalar.activation(zt[ti][:el, :], zt[ti][:el, :], AF.Silu)
                    nc.vector.tensor_tensor(y_fm[ti][:el, :], y_fm[ti][:el, :],
                                            zt[ti][:el, :], OP.mult)

                # ---- out_proj partial + AllReduce + residual update ----
                op_in = dramp.tile([DM, T], BF16, name="op_in", tag="op_in")
                op_out = dramp.tile([DM, T], BF16, name="op_out", tag="op_out")
                for mt in range(DTILES):
                    for f in range(T // 512):
                        fs = slice(f * 512, (f + 1) * 512)
                        pt = pmm([128, 512])
                        for ti, (eo, el) in enumerate(_etiles()):
                            nc.tensor.matmul(
                                pt[:], w_out[0:el, ti * DM + mt * 128:ti * DM + (mt + 1) * 128],
                                y_fm[ti][:el, fs], start=(ti == 0), stop=(ti == 1))
                        opf = st2.tile([128, 512], BF16, name="opf", tag="opf", bufs=2)
                        nc.scalar.copy(opf[:], pt[:])
                        nc.sync.dma_start(op_in[mt * 128:(mt + 1) * 128, fs], opf[:])
                nc.gpsimd.collective_compute("AllReduce", OP.add,
                                             replica_groups=[list(range(NC))],
                                             ins=[op_in[:]], outs=[op_out[:]])
                for j in range(DTILES):
                    for f in range(T // 512):
                        fs = slice(f * 512, (f + 1) * 512)
                        hs_f = st2.tile([128, 512], BF16, name="hs_f", tag="hs_f")
                        nc.sync.dma_start(hs_f[:], op_out[j * 128:(j + 1) * 128, fs])
                        nc.vector.tensor_tensor(residual[j][:, fs], residual[j][:, fs],
                                                hs_f[:], OP.add)

            # ================= final stage =================
            mixed = [actp.tile([128, T], BF16, name=f"mx{j}", tag=t)
                     for j, t in enumerate(
                         ["xp_pad0", "xc0", "yfm0", "dA", "dstage", "u16_0"])]

            def consume_mixed(f, sl6):
                fs = slice(f * 512, (f + 1) * 512)
                for j in range(DTILES):
                    nc.vector.tensor_copy(mixed[j][:, fs], sl6[j][:])

            ln_fm([nrmc[:, 2 * j:2 * j + 1] for j in range(DTILES)],
                  [nrmc[:, 2 * j + 1:2 * j + 2] for j in range(DTILES)],
                  consume_mixed)

            brow = actp.tile([1, T], BF16, name="brow", tag="stats16")
            nc.sync.dma_start(brow[:], bprob_d[:])

            wc1 = wpool.tile([128, 7 * GDM], BF16, name="wc1", tag="w_in_sb")
            nc.sync.dma_start(wc1[:], w_c1_d[:])
            bc1 = wpool.tile([GDM, 1], FP32, name="bc1", tag="w_cb_sb")
            nc.sync.dma_start(bc1[:], b_c1_d[:])
            wc2 = wpool.tile([GDM + 1, DM], BF16, name="wc2", tag="w_out_sb")
            nc.sync.dma_start(wc2[:], w_c2_d[:])

            h1 = actp.tile([GDM + 1, T], BF16, name="h1", tag="dtf16")
            nc.vector.memset(h1[GDM:GDM + 1, :], 1.0)
            for f in range(T // 512):
                fs = slice(f * 512, (f + 1) * 512)
                xfb = [st2.tile([128, 512], BF16, name=f"xfb{j}", tag=f"hlnf{j}")
                       for j in range(DTILES)]
                for c4 in range(4):
                    c = f * 4 + c4
                    x_tm_c = st2.tile([128, DM], FP32, name="x_tm_c3", tag="x_tm_c")
                    nc.sync.dma_start(x_tm_c[:], x_d[c * Q:(c + 1) * Q, :])
                    for j in range(DTILES):
                        ptt = pmm([128, 128])
                        nc.tensor.transpose(ptt[:], x_tm_c[:, j * 128:(j + 1) * 128],
                                            ident32[:])
                        nc.scalar.copy(xfb[j][:, c4 * 128:(c4 + 1) * 128], ptt[:])
                pt = pmm([GDM, 512])
                for kt in range(DTILES):
                    nc.tensor.matmul(pt[:], wc1[:, kt * GDM:(kt + 1) * GDM],
                                     xfb[kt][:], start=(kt == 0), stop=False)
                nc.tensor.matmul(pt[:], wc1[0:1, 6 * GDM:7 * GDM], brow[:, fs],
                                 start=False, stop=True)
                nc.scalar.activation(h1[0:GDM, fs], pt[:], AF.Silu, bias=bc1[:, 0:1])

            g_in = dramp.tile([T, DM], FP32, name="g_in", tag="g_in")
            g_out = dramp.tile([T, DM], FP32, name="g_out", tag="g_out")
            for c in range(NCH):
                h2sb = st2.tile([128, DM], FP32, name="h2sb", tag="h2sb")
                for fs2 in range(2):
                    pt = pmm([128, 384])
                    nc.tensor.matmul(pt[:], h1[:, c * Q:(c + 1) * Q],
                                     wc2[:, fs2 * 384:(fs2 + 1) * 384],
                                     start=True, stop=True)
                    nc.scalar.copy(h2sb[:, fs2 * 384:(fs2 + 1) * 384], pt[:])
                nc.sync.dma_start(g_in[c * Q:(c + 1) * Q, :], h2sb[:])
            nc.gpsimd.collective_compute("AllReduce", OP.add,
                                         replica_groups=[list(range(NC))],
                                         ins=[g_in[:]], outs=[g_out[:]])

            n16 = actp.tile([1, DM], BF16, name="n16", tag="n16")
            n16b = actp.tile([1, DM], BF16, name="n16b", tag="n16b")
            nr32 = st2.tile([1, DM], FP32, name="nr32", tag="h2sb")
            nc.sync.dma_start(nr32[:], nrm_d[2:3, :])
            nc.vector.tensor_copy(n16[:], nr32[:])
            nr32b = st2.tile([1, DM], FP32, name="nr32b", tag="h2sb")
            nc.sync.dma_start(nr32b[:], nrm_d[3:4, :])
            nc.vector.tensor_copy(n16b[:], nr32b[:])
            nfw_rep = actp.tile([128, DM], BF16, name="nfw_rep", tag="nfw_rep")
            nfb_rep = actp.tile([128, DM], BF16, name="nfb_rep", tag="nfb_rep")
            for fs2 in range(2):
                rp = pmm([128, 384])
                nc.tensor.matmul(rp[:], onesrow16[:1, :128],
                                 n16[0:1, fs2 * 384:(fs2 + 1) * 384], start=True, stop=True)
                nc.scalar.copy(nfw_rep[:, fs2 * 384:(fs2 + 1) * 384], rp[:])
                rp2 = pmm([128, 384])
                nc.tensor.matmul(rp2[:], onesrow16[:1, :128],
                                 n16b[0:1, fs2 * 384:(fs2 + 1) * 384], start=True, stop=True)
                nc.scalar.copy(nfb_rep[:, fs2 * 384:(fs2 + 1) * 384], rp2[:])

            for c in range(NCH):
                mixed_tm = st2.tile([128, DM], BF16, name="mixed_tm", tag="mixed_tm")
                for j in range(DTILES):
                    ptt = pmm([128, 128], BF16)
                    nc.tensor.transpose(ptt[:], mixed[j][:, c * Q:(c + 1) * Q], ident16[:])
                    nc.scalar.copy(mixed_tm[:, j * 128:(j + 1) * 128], ptt[:])
                xt = st2.tile([128, DM], FP32, name="xt", tag="x_tm_c")
                nc.sync.dma_start(xt[:], x_d[c * Q:(c + 1) * Q, :])
                gt = st2.tile([128, DM], FP32, name="gt", tag="cacc", bufs=1)
                nc.sync.dma_start(gt[:], g_out[c * Q:(c + 1) * Q, :])
                nc.scalar.activation(gt[:], gt[:], AF.Sigmoid)
                nc.sync.dma_start(gate_d[c * Q:(c + 1) * Q, :], gt[:])
                ot = st2.tile([128, DM], FP32, name="ot", tag="ot", bufs=1)
                nc.vector.tensor_tensor(ot[:], mixed_tm[:], xt[:], OP.subtract)
                nc.vector.tensor_tensor(ot[:], ot[:], gt[:], OP.mult)
                nc.vector.tensor_tensor(ot[:], ot[:], xt[:], OP.add)
                st = st2.tile([128, 1], FP32, name="st", tag="st")
                nc.vector.tensor_reduce(st[:], ot[:], axis=AX.X, op=OP.add)
                nc.scalar.activation(st[:], st[:], AF.Copy, scale=1.0 / DM)
                nc.vector.tensor_scalar(ot[:], ot[:], st[:, 0:1], None, OP.subtract)
                sq2 = st2.tile([128, DM], FP32, name="sq2", tag="h2sb")
                nc.vector.tensor_tensor(sq2[:], ot[:], ot[:], OP.mult)
                v2 = st2.tile([128, 1], FP32, name="v2", tag="v2")
                nc.vector.tensor_reduce(v2[:], sq2[:], axis=AX.X, op=OP.add)
                nc.scalar.activation(v2[:], v2[:], AF.Ln, bias=eps_ap[:], scale=1.0 / DM)
                nc.scalar.activation(v2[:], v2[:], AF.Exp, scale=-0.5)
                nc.vector.tensor_scalar(ot[:], ot[:], v2[:, 0:1], None, OP.mult)
                nc.vector.tensor_tensor(ot[:], ot[:], nfw_rep[:], OP.mult)
                nc.vector.tensor_tensor(ot[:], ot[:], nfb_rep[:], OP.add)
                nc.sync.dma_start(out_d[c * Q:(c + 1) * Q, :], ot[:])

    nc.compile()
    return nc


def _pack_fm(arr, pad_to=128):
    arr = np.asarray(arr)
    if arr.ndim == 1:
        arr = arr[:, None]
    F, W = arr.shape
    nblk = (F + pad_to - 1) // pad_to
    outp = np.zeros((pad_to, nblk * W), dtype=arr.dtype)
    for b_ in range(nblk):
        blk = arr[b_ * pad_to:(b_ + 1) * pad_to]
        outp[:blk.shape[0], b_ * W:(b_ + 1) * W] = blk
    return outp


def _prep_inputs(inputs):
    f32 = np.float32
    x = np.ascontiguousarray(np.asarray(inputs["x"], f32).reshape(T, DM))
    bprob = np.ascontiguousarray(np.asarray(inputs["boundary_prob"], f32).reshape(1, T))
    A = -np.exp(np.asarray(inputs["A_log"], f32))
    a_scales = A[0, 0, :]

    r = np.arange(128)
    # REPA[e', 128k + r] = (e' == 8k + r%8) for k<16 (contraction over e' in [0,128))
    repa = np.zeros((128, 16 * 128), f32)
    for k in range(16):
        repa[:, 128 * k:128 * (k + 1)] = (np.arange(128)[:, None] ==
                                          (8 * k + (r % 8))[None, :])
    repb = np.zeros((64, 8 * 128), f32)
    for kk in range(8):
        repb[:, 128 * kk:128 * (kk + 1)] = (np.arange(64)[:, None] ==
                                            (8 * kk + (r % 8))[None, :])
    # POOL[r, 128k + j] = (j == 8*(k%16) + r%8)
    poolm = np.zeros((128, NKT * 128), f32)
    for k in range(NKT):
        poolm[:, 128 * k:128 * (k + 1)] = ((8 * (k % 16) + (r % 8))[:, None] ==
                                           np.arange(128)[None, :])
    # REPN[n, r] = (n == r//8)   (Brep/Crep: B16 row n -> rows 8n..8n+7)
    repn = (np.arange(16)[:, None] == (r // 8)[None, :]).astype(f32)
    acol = a_scales[(r // 8) % 16].astype(f32)[:, None]

    maps = []
    for c in range(NC):
        sl = slice(c * ELOC, (c + 1) * ELOC)
        w_in = np.stack([_pack_fm(
            np.concatenate([np.asarray(inputs["in_proj_w"][i])[sl],
                            np.asarray(inputs["in_proj_w"][i])[E + c * ELOC:E + (c + 1) * ELOC]],
                           axis=0).T.astype(f32))
            for i in range(NL)])
        w_xp = np.stack([_pack_fm(np.asarray(inputs["x_proj_w"][i], f32)[:, sl].T)
                         for i in range(NL)])
        w_dt = np.stack([np.asarray(inputs["dt_proj_w"][i], f32)[sl].T
                         for i in range(NL)])
        dtb = np.stack([_pack_fm(np.asarray(inputs["dt_proj_b"][i], f32)[sl])
                        for i in range(NL)])
        w_out = np.stack([_pack_fm(np.asarray(inputs["out_proj_w"][i], f32)[:, sl].T)
                          for i in range(NL)])
        lnp = np.stack([_pack_fm(np.stack([np.asarray(inputs["ln_w"][i], f32),
                                           np.asarray(inputs["ln_b"][i], f32)], axis=1))
                        for i in range(NL)])
        gsl = slice(c * GDM, (c + 1) * GDM)
        cw1 = np.asarray(inputs["ctrl_w1"], f32)
        w_c1 = np.concatenate([_pack_fm(cw1[gsl, :DM].T),
                               _pack_fm(cw1[gsl, DM:DM + 1].T)], axis=1)
        w_c2 = np.concatenate([np.asarray(inputs["ctrl_w2"], f32)[:, gsl].T,
                               (np.asarray(inputs["ctrl_b2"], f32) / NC)[None, :]], axis=0)
        nrm = np.stack([np.asarray(inputs["normf_w"], f32), np.asarray(inputs["normf_b"], f32),
                        np.asarray(inputs["out_ln_w"], f32), np.asarray(inputs["out_ln_b"], f32)])
        nrmc = _pack_fm(np.stack([np.asarray(inputs["normf_w"], f32),
                                  np.asarray(inputs["normf_b"], f32)], axis=1))
        maps.append({
            "x": x, "bprob": bprob, "w_in": w_in,
            "conv_w": np.stack([_pack_fm(np.asarray(inputs["conv_w"][i], f32)[sl])
                                for i in range(NL)]),
            "conv_b": np.stack([_pack_fm(np.asarray(inputs["conv_b"][i], f32)[sl])
                                for i in range(NL)]),
            "w_xp": w_xp, "w_dt": w_dt, "dtb": dtb, "w_out": w_out, "lnp": lnp,
            "ssmd": np.stack([_pack_fm(np.asarray(inputs["ssm_D"][i], f32)[sl])
                              for i in range(NL)]),
            "w_c1": w_c1,
            "b_c1": np.asarray(inputs["ctrl_b1"], f32)[gsl][:, None],
            "w_c2": w_c2, "nrm": nrm, "nrmc": nrmc,
            "repa": repa, "repb": repb, "poolm": poolm, "repn": repn,
            "acol": acol,
        })
    return maps


def _cast_bf16(maps):
    import ml_dtypes
    for m in maps:
        for k in ("w_in", "w_xp", "w_dt", "w_out", "w_c1", "w_c2", "bprob",
                  "repa", "repb", "poolm", "repn"):
            m[k] = np.asarray(m[k], dtype=ml_dtypes.bfloat16)
    return maps


def kernel(**inputs):
    maps = _prep_inputs(inputs)
    A = -np.exp(np.asarray(inputs["A_log"], np.float32))
    a_scales = A[0, 0, :]
    for i in range(NL):
        assert np.allclose(A[i], np.broadcast_to(a_scales, (E, N)), rtol=1e-5, atol=1e-6), \
            "kernel assumes channel-independent A"
    if "nc" not in _CACHE:
        _CACHE["nc"] = _build()
    nc = _CACHE["nc"]
    _cast_bf16(maps)
    res = run_bass_kernel_spmd(nc, maps, list(range(NC)))
    kernel._res = res
    r0 = res.results[0]
    out = np.asarray(r0["out"], np.float32).reshape(B, L, DM)
    gate = np.asarray(r0["gate"], np.float32).reshape(B, L, DM)
    return out, gate



# revision 14
# speedup vs baseline: 1.6097x; 1.6097x over previous
"""Trainium2 Bass kernel for nn_BoundaryControlledMixer (4-layer Mamba stack +
boundary-controlled gate), tensor-parallel over d_inner across 8 NeuronCores.

Per core (owns E_loc = 192 of E = 1536 channels, full batch/sequence):
  - Activations flow feature-major [feat, token] so projections chain on the
    PE without transposes (matmul contracts the partition dim).
  - Selective scan: DVE tensor_tensor_scan (state = dA*state + b, fp32 state)
    over 24 row-tiles of the (n, e) grid: tile k holds rows r = n*8 + e_l,
    e = 8k + e_l, n = r//8, scanned along the full token axis [128, T].
    Batch reset via dA[:, L] = 0.  Inputs built by PE 0/1-replication
    (dt -> dtrep -> exp = dA) and log-doubling DMA (u = dt*xc -> ubrep),
    b = ubrep * Brep.  y = sum_n C*h via a 0/1 pooling matmul.
"""

import numpy as np

import concourse.bacc as bacc
import concourse.bass as bass
import concourse.mybir as mybir
import concourse.tile as tile
from concourse import masks
from concourse.bass_utils import run_bass_kernel_spmd

FP32 = mybir.dt.float32
BF16 = mybir.dt.bfloat16
AF = mybir.ActivationFunctionType
OP = mybir.AluOpType
AX = mybir.AxisListType

B, L, DM, NL = 2, 1024, 768, 4
E, N, K, R = 2 * DM, 16, 4, DM // 16
NC = 8
ELOC = E // NC            # 192
T = B * L                 # 2048
Q = 128
NCH = T // Q              # 16
EPS = 1e-5
DTILES = DM // 128        # 6
GDM = DM // NC            # 96
LPD = L + 2 * K           # padded per-batch xp row
NKT = N * ELOC // 128     # 24 scan tiles
RD = R + 1                # 49

_CACHE = {}
_DEBUG = False


def _etiles():
    return [(0, 128), (128, 64)]


def _build():
    nc = bacc.Bacc("TRN2", target_bir_lowering=False, debug=False)

    x_d = nc.dram_tensor("x", [T, DM], FP32, kind="ExternalInput")
    bprob_d = nc.dram_tensor("bprob", [1, T], BF16, kind="ExternalInput")
    w_in_d = nc.dram_tensor("w_in", [NL, 128, 6 * 2 * ELOC], BF16, kind="ExternalInput")
    conv_w_d = nc.dram_tensor("conv_w", [NL, 128, 2 * K], FP32, kind="ExternalInput")
    conv_b_d = nc.dram_tensor("conv_b", [NL, 128, 2], FP32, kind="ExternalInput")
    w_xp_d = nc.dram_tensor("w_xp", [NL, 128, 2 * (R + 2 * N)], BF16, kind="ExternalInput")
    w_dt_d = nc.dram_tensor("w_dt", [NL, R, ELOC], BF16, kind="ExternalInput")
    dtb_d = nc.dram_tensor("dtb", [NL, 128, 2], FP32, kind="ExternalInput")
    w_out_d = nc.dram_tensor("w_out", [NL, 128, 2 * DM], BF16, kind="ExternalInput")
    ln_d = nc.dram_tensor("lnp", [NL, 128, 12], FP32, kind="ExternalInput")
    ssmd_d = nc.dram_tensor("ssmd", [NL, 128, 2], FP32, kind="ExternalInput")
    w_c1_d = nc.dram_tensor("w_c1", [128, 7 * GDM], BF16, kind="ExternalInput")
    b_c1_d = nc.dram_tensor("b_c1", [GDM, 1], FP32, kind="ExternalInput")
    w_c2_d = nc.dram_tensor("w_c2", [GDM + 1, DM], BF16, kind="ExternalInput")
    nrm_d = nc.dram_tensor("nrm", [4, DM], FP32, kind="ExternalInput")
    nrmc_d = nc.dram_tensor("nrmc", [128, 12], FP32, kind="ExternalInput")
    repa_d = nc.dram_tensor("repa", [128, 16 * 128], BF16, kind="ExternalInput")
    repb_d = nc.dram_tensor("repb", [64, 8 * 128], BF16, kind="ExternalInput")
    pool_d = nc.dram_tensor("poolm", [128, NKT * 128], BF16, kind="ExternalInput")
    repn_d = nc.dram_tensor("repn", [16, 128], BF16, kind="ExternalInput")
    acol_d = nc.dram_tensor("acol", [128, 1], FP32, kind="ExternalInput")

    out_d = nc.dram_tensor("out", [T, DM], FP32, kind="ExternalOutput")
    gate_d = nc.dram_tensor("gate", [T, DM], FP32, kind="ExternalOutput")
    dbg = {}
    if _DEBUG:
        dbg["dt"] = nc.dram_tensor("dbg_dt", [ELOC, T], FP32, kind="ExternalOutput")
        dbg["yssm"] = nc.dram_tensor("dbg_yssm", [ELOC, T], FP32, kind="ExternalOutput")
        dbg["xc"] = nc.dram_tensor("dbg_xc", [ELOC, T], FP32, kind="ExternalOutput")
        dbg["hln"] = nc.dram_tensor("dbg_hln", [DM, T], FP32, kind="ExternalOutput")

    with tile.TileContext(nc) as tc:
        with tc.tile_pool(name="const", bufs=1) as constp, \
             tc.tile_pool(name="persist", bufs=1) as pers, \
             tc.tile_pool(name="wts", bufs=1) as wpool, \
             tc.tile_pool(name="act", bufs=1) as actp, \
             tc.tile_pool(name="st2", bufs=1) as st2, \
             tc.tile_pool(name="scn", bufs=1) as scn, \
             tc.tile_pool(name="ps_rep", bufs=1, space="PSUM") as ps_rep, \
             tc.tile_pool(name="ps_pool", bufs=1, space="PSUM") as ps_pool, \
             tc.tile_pool(name="ps_mm", bufs=2, space="PSUM") as ps_mm, \
             tc.tile_pool(name="dram", bufs=2, space="DRAM") as dramp:

            def pmm(shape, dt=FP32):
                return ps_mm.tile(shape, dt, name="pmm", tag="pmm")

            # ---------- constants ----------
            ident32 = constp.tile([128, 128], FP32)
            masks.make_identity(nc, ident32[:])
            ident16 = constp.tile([128, 128], BF16)
            masks.make_identity(nc, ident16[:])
            onesrow16 = constp.tile([1, 512], BF16)
            nc.gpsimd.memset(onesrow16[:], 1.0)
            halfcol32 = constp.tile([128, 1], FP32)
            nc.gpsimd.memset(halfcol32[:], 0.5)
            halfcol16 = constp.tile([128, 1], BF16)
            nc.gpsimd.memset(halfcol16[:], 0.5)
            eps_ap = constp.tile([128, 1], FP32)
            nc.gpsimd.memset(eps_ap[:], EPS)
            nrmc = constp.tile([128, 12], FP32)
            nc.sync.dma_start(nrmc[:], nrmc_d[:])
            repa = constp.tile([128, 16 * 128], BF16)
            nc.sync.dma_start(repa[:], repa_d[:])
            repb = constp.tile([64, 8 * 128], BF16)
            nc.sync.dma_start(repb[:], repb_d[:])
            poolm = constp.tile([128, NKT * 128], BF16)
            nc.sync.dma_start(poolm[:], pool_d[:])
            repn = constp.tile([16, 128], BF16)
            nc.sync.dma_start(repn[:], repn_d[:])
            acol = constp.tile([128, 1], FP32)
            nc.sync.dma_start(acol[:], acol_d[:])

            # ---------- x -> feature-major fp32 residual ----------
            residual = [pers.tile([128, T], FP32, name=f"res{j}") for j in range(DTILES)]
            for c in range(NCH):
                x_tm_c = st2.tile([128, DM], FP32, name="x_tm_c", tag="x_tm_c")
                nc.sync.dma_start(x_tm_c[:], x_d[c * Q:(c + 1) * Q, :])
                for j in range(DTILES):
                    pt = pmm([128, 128])
                    nc.tensor.transpose(pt[:], x_tm_c[:, j * 128:(j + 1) * 128], ident32[:])
                    nc.scalar.copy(residual[j][:, c * Q:(c + 1) * Q], pt[:])

            # ---------- fused feature-major LayerNorm ----------
            def ln_fm(lnw_aps, lnb_aps, consume, dbg_dst=None):
                stats16 = actp.tile([1, T], BF16, name="stats16", tag="stats16")
                stats16b = actp.tile([1, T], BF16, name="stats16b", tag="stats16b")

                for f in range(T // 512):
                    fs = slice(f * 512, (f + 1) * 512)
                    sp1 = pmm([1, 512])
                    sp2 = pmm([1, 512])
                    for j in range(DTILES):
                        nc.tensor.matmul(sp1[:], halfcol32, residual[j][:, fs],
                                         start=(j == 0), stop=(j == DTILES - 1))
                    nc.scalar.activation(stats16[0:1, fs], sp1[:], AF.Copy, scale=2.0 / DM)
                    for j in range(DTILES):
                        sqj = st2.tile([128, 512], BF16, name="ln_sqj", tag="ln_sqj")
                        nc.vector.tensor_tensor(sqj[:], residual[j][:, fs],
                                                residual[j][:, fs], OP.mult)
                        nc.tensor.matmul(sp2[:], halfcol16[:], sqj[:],
                                         start=(j == 0), stop=(j == DTILES - 1))
                    nc.scalar.activation(stats16b[0:1, fs], sp2[:], AF.Copy, scale=2.0 / DM)

                for f in range(T // 512):
                    fs = slice(f * 512, (f + 1) * 512)
                    rp = pmm([128, 512])
                    nc.tensor.matmul(rp[:], onesrow16[:1, :128], stats16[0:1, fs],
                                     start=True, stop=True)
                    meanr = st2.tile([128, 512], FP32, name="ln_meanr", tag="ln_meanr")
                    nc.scalar.copy(meanr[:], rp[:])
                    rp2 = pmm([128, 512])
                    nc.tensor.matmul(rp2[:], onesrow16[:1, :128], stats16b[0:1, fs],
                                     start=True, stop=True)
                    invr = st2.tile([128, 512], FP32, name="ln_invr", tag="ln_invr")
                    nc.scalar.copy(invr[:], rp2[:])
                    # var = E[x^2] - mean^2 ; inv = exp(-0.5*ln(var+eps))
                    c2r = st2.tile([128, 512], FP32, name="ln_c2r", tag="ln_c2r")
                    nc.vector.tensor_tensor(c2r[:], meanr[:], meanr[:], OP.mult)
                    nc.vector.tensor_tensor(invr[:], invr[:], c2r[:], OP.subtract)
                    nc.scalar.activation(invr[:], invr[:], AF.Ln, bias=eps_ap[:])
                    nc.scalar.activation(invr[:], invr[:], AF.Exp, scale=-0.5)
                    nc.vector.tensor_tensor(c2r[:], meanr[:], invr[:], OP.mult)
                    slices = []
                    for j in range(DTILES):
                        tmp = st2.tile([128, 512], BF16, name="ln_tmp", tag="ln_tmp", bufs=2)
                        nc.vector.tensor_tensor(tmp[:], residual[j][:, fs], invr[:], OP.mult)
                        nc.vector.tensor_tensor(tmp[:], tmp[:], c2r[:], OP.subtract)
                        hlnf = st2.tile([128, 512], BF16, name="hlnf", tag=f"hlnf{j}")
                        nc.scalar.activation(hlnf[:], tmp[:], AF.Identity,
                                             scale=lnw_aps[j], bias=lnb_aps[j])
                        slices.append(hlnf)
                        if dbg_dst is not None:
                            hld = st2.tile([128, 512], FP32, name="hld", tag="hld")
                            nc.vector.tensor_copy(hld[:], hlnf[:])
                            nc.sync.dma_start(dbg_dst[j * 128:(j + 1) * 128, fs], hld[:])
                    consume(f, slices)

            # ================= layers =================
            for li in range(NL):
                w_in = wpool.tile([128, 6 * 2 * ELOC], BF16, name="w_in_sb", tag="w_in_sb")
                nc.sync.dma_start(w_in[:], w_in_d[li])
                w_cw = wpool.tile([128, 2 * K], FP32, name="w_cw_sb", tag="w_cw_sb")
                nc.sync.dma_start(w_cw[:], conv_w_d[li])
                w_cb = wpool.tile([128, 2], FP32, name="w_cb_sb", tag="w_cb_sb")
                nc.sync.dma_start(w_cb[:], conv_b_d[li])
                w_xp = wpool.tile([128, 2 * (R + 2 * N)], BF16, name="w_xp_sb", tag="w_xp_sb")
                nc.sync.dma_start(w_xp[:], w_xp_d[li])
                w_dt = wpool.tile([R, ELOC], BF16, name="w_dt_sb", tag="w_dt_sb")
                nc.sync.dma_start(w_dt[:], w_dt_d[li])
                dtb = wpool.tile([128, 2], FP32, name="dtb_sb", tag="dtb_sb")
                nc.sync.dma_start(dtb[:], dtb_d[li])
                w_out = wpool.tile([128, 2 * DM], BF16, name="w_out_sb", tag="w_out_sb")
                nc.sync.dma_start(w_out[:], w_out_d[li])
                w_ln = wpool.tile([128, 12], FP32, name="w_ln_sb", tag="w_ln_sb")
                nc.sync.dma_start(w_ln[:], ln_d[li])
                w_D = wpool.tile([128, 2], FP32, name="w_D_sb", tag="w_D_sb")
                nc.sync.dma_start(w_D[:], ssmd_d[li])

                # ---- LN fused with in_proj ----
                xp_t = [actp.tile([128, B * LPD], BF16, name="xp_pad0", tag="xp_pad0"),
                        actp.tile([64, B * LPD], BF16, name="xp_pad1", tag="xp_pad1")]
                z_sb = [actp.tile([128, T], BF16, name="z_sb0", tag="z_sb0"),
                        actp.tile([64, T], BF16, name="z_sb1", tag="z_sb1")]
                for ti in range(2):
                    nc.vector.memset(xp_t[ti][:, 0:K], 0.0)
                    nc.vector.memset(xp_t[ti][:, LPD:LPD + K], 0.0)

                def padcol(fs, fl):
                    b_ = fs // L
                    off = b_ * LPD + K + (fs - b_ * L)
                    return slice(off, off + fl)

                def consume_inproj(f, sl6):
                    fs = f * 512
                    for mt in range(3):
                        pt = pmm([128, 512])
                        for kt in range(DTILES):
                            nc.tensor.matmul(
                                pt[:], w_in[:, kt * 384 + mt * 128:kt * 384 + (mt + 1) * 128],
                                sl6[kt][:], start=(kt == 0), stop=(kt == DTILES - 1))
                        if mt == 0:
                            nc.scalar.copy(xp_t[0][:, padcol(fs, 512)], pt[:])
                        elif mt == 1:
                            nc.scalar.copy(xp_t[1][:, padcol(fs, 512)], pt[0:64, :])
                            nc.scalar.copy(z_sb[0][0:64, fs:fs + 512], pt[64:128, :])
                        else:
                            nc.scalar.copy(z_sb[0][64:128, fs:fs + 512], pt[0:64, :])
                            nc.scalar.copy(z_sb[1][0:64, fs:fs + 512], pt[64:128, :])

                ln_fm([w_ln[:, 2 * j:2 * j + 1] for j in range(DTILES)],
                      [w_ln[:, 2 * j + 1:2 * j + 2] for j in range(DTILES)],
                      consume_inproj,
                      dbg_dst=dbg["hln"] if (_DEBUG and li == 0) else None)

                # ---- conv + silu ----
                xc = [actp.tile([128, T], BF16, name="xc0", tag="xc0"),
                      actp.tile([64, T], BF16, name="xc1", tag="xc1")]
                for ti, (eo, el) in enumerate(_etiles()):
                    for b_ in range(B):
                        acc = st2.tile([el, L], FP32, name="cacc", tag="cacc", bufs=1)
                        cb = b_ * LPD + K
                        nc.vector.tensor_scalar(acc[:], xp_t[ti][:el, cb - 3:cb - 3 + L],
                                                w_cw[0:el, ti * K:ti * K + 1], None, OP.mult)
                        for j in range(1, K):
                            nc.vector.scalar_tensor_tensor(
                                acc[:], xp_t[ti][:el, cb - 3 + j:cb - 3 + j + L],
                                w_cw[0:el, ti * K + j:ti * K + j + 1],
                                acc[:], OP.mult, OP.add)
                        nc.scalar.activation(xc[ti][:el, b_ * L:(b_ + 1) * L], acc[:],
                                             AF.Silu, bias=w_cb[0:el, ti:ti + 1])
                if _DEBUG and li == 0:
                    for ti, (eo, el) in enumerate(_etiles()):
                        xcd = st2.tile([el, T], FP32, name="xcd", tag="xcd")
                        nc.vector.tensor_copy(xcd[:], xc[ti][:el, :])
                        nc.sync.dma_start(dbg["xc"][eo:eo + el, :], xcd[:])

                # ---- x_proj partial + AllReduce (bf16 wire, Shared out) ----
                dbl_in = dramp.tile([R + 2 * N, T], BF16, name="dbl_in", tag="dbl_in")
                dbl_out = dramp.tile([R + 2 * N, T], BF16, name="dbl_out",
                                     tag="dbl_out", addr_space="Shared", bufs=NL)
                for f in range(T // 512):
                    fs = slice(f * 512, (f + 1) * 512)
                    pt = pmm([80, 512])
                    for ti, (eo, el) in enumerate(_etiles()):
                        nc.tensor.matmul(pt[:], w_xp[0:el, ti * 80:(ti + 1) * 80],
                                         xc[ti][:el, fs], start=(ti == 0), stop=(ti == 1))
                    dblf = st2.tile([80, 512], BF16, name="dblf", tag="dblf")
                    nc.scalar.copy(dblf[:], pt[:])
                    nc.sync.dma_start(dbl_in[:, fs], dblf[:])
                nc.gpsimd.collective_compute("AllReduce", OP.add,
                                             replica_groups=[list(range(NC))],
                                             ins=[dbl_in[:]], outs=[dbl_out[:]])

                # ---- dt path: softplus(w_dt @ dbl + b) e-major ----
                dtf16 = scn.tile([R, T], BF16, name="dtf16", tag="dtf16", bufs=2)
                nc.sync.dma_start(dtf16[:], dbl_out[0:R, :])

                dt16 = [scn.tile([128, T], BF16, name="dt16_0", tag="dt16_0"),
                        scn.tile([64, T], BF16, name="dt16_1", tag="dt16_1")]
                for ti, (eo, el) in enumerate(_etiles()):
                    for h_ in range(2):
                        hsl = slice(h_ * 1024, (h_ + 1) * 1024)
                        ptd = ps_rep.tile([128, 1024], FP32, name="ptd", tag="rep",
                                          bufs=2)
                        for cc in range(2):
                            csl = slice(h_ * 1024 + cc * 512, h_ * 1024 + (cc + 1) * 512)
                            psl = slice(cc * 512, (cc + 1) * 512)
                            nc.tensor.matmul(ptd[0:el, psl],
                                             w_dt[:, eo:eo + el], dtf16[:, csl],
                                             start=True, stop=True)
                        e1 = scn.tile([el, 1024], FP32, name="e1", tag="dstage")
                        nc.scalar.activation(e1[:], ptd[0:el, :], AF.Exp,
                                             bias=dtb[0:el, ti:ti + 1])
                        nc.scalar.activation(dt16[ti][:el, hsl], e1[:], AF.Ln, bias=1.0)
                if _DEBUG and li == 0:
                    dtd = st2.tile([128, T], FP32, name="dtd", tag="dtd")
                    nc.vector.tensor_copy(dtd[:], dt16[0][:])
                    nc.sync.dma_start(dbg["dt"][0:128, :], dtd[:])
                    dtd2 = st2.tile([64, T], FP32, name="dtd2", tag="dtd")
                    nc.vector.tensor_copy(dtd2[:], dt16[1][:])
                    nc.sync.dma_start(dbg["dt"][128:192, :], dtd2[:])

                # ---- u = dt * xc ; B/C replication ----
                u16 = [scn.tile([128, T], BF16, name="u16_0", tag="u16_0"),
                       scn.tile([64, T], BF16, name="u16_1", tag="u16_1")]
                for ti, (eo, el) in enumerate(_etiles()):
                    nc.vector.tensor_tensor(u16[ti][:el, :], dt16[ti][:el, :],
                                            xc[ti][:el, :], OP.mult)

                Brep = scn.tile([128, T], BF16, name="Brep", tag="Brep")
                Crep = scn.tile([128, T], BF16, name="Crep", tag="Crep")
                for si, dst in ((0, Brep), (1, Crep)):
                    src16 = scn.tile([16, T], BF16, name=f"bc16_{si}", tag="dtf16", bufs=2)
                    nc.sync.dma_start(src16[:], dbl_out[R + si * N:R + (si + 1) * N, :])
                    for h_ in range(2):
                        hsl = slice(h_ * 1024, (h_ + 1) * 1024)
                        prr = ps_rep.tile([128, 1024], FP32, name="prr", tag="rep",
                                          bufs=2)
                        for cc in range(2):
                            psl = slice(cc * 512, (cc + 1) * 512)
                            csl = slice(h_ * 1024 + cc * 512, h_ * 1024 + (cc + 1) * 512)
                            nc.tensor.matmul(prr[:, psl], repn[:], src16[:, csl],
                                             start=True, stop=True)
                        nc.scalar.copy(dst[:, hsl], prr[:])

                # ---- scan tiles ----
                # Per (group, batch): pool accumulator [128, L] (2 PSUM banks);
                # dt/u replication via PE matmul into [128, L] PSUM (2 banks,
                # bufs=2), consumed by scalar (exp -> dA, copy -> urep).
                y_fm = [actp.tile([128, T], BF16, name="yfm0", tag="xp_pad0"),
                        actp.tile([64, T], BF16, name="yfm1", tag="xp_pad1")]
                for g, (ks, el_in) in enumerate(((range(0, 16), 128),
                                                 (range(16, 24), 64))):
                    rows_out = 128 if g == 0 else 64
                    repm = repa if g == 0 else repb
                    for b_ in range(B):
                        bsl = slice(b_ * L, (b_ + 1) * L)
                        pp = ps_pool.tile([128, L], FP32, name="pp", tag="pp")
                        for k in ks:
                            kk = k - 16 * g
                            # dt replication -> dA = exp(a * dtrep)
                            prp = ps_rep.tile([128, L], FP32, name="prp", tag="rep",
                                              bufs=2)
                            for cc in range(2):
                                psl = slice(cc * 512, (cc + 1) * 512)
                                csl = slice(b_ * L + cc * 512, b_ * L + (cc + 1) * 512)
                                nc.tensor.matmul(prp[:, psl],
                                                 repm[:, kk * 128:(kk + 1) * 128],
                                                 dt16[g][:el_in, csl],
                                                 start=True, stop=True)
                            dA = scn.tile([128, L], FP32, name="dA", tag="dA", bufs=2)
                            nc.scalar.activation(dA[:], prp[:], AF.Exp,
                                                 scale=acol[:, 0:1])
                            # u replication via PE (same 0/1 matrix)
                            pru = ps_rep.tile([128, L], FP32, name="pru", tag="rep",
                                              bufs=2)
                            for cc in range(2):
                                psl = slice(cc * 512, (cc + 1) * 512)
                                csl = slice(b_ * L + cc * 512, b_ * L + (cc + 1) * 512)
                                nc.tensor.matmul(pru[:, psl],
                                                 repm[:, kk * 128:(kk + 1) * 128],
                                                 u16[g][:el_in, csl],
                                                 start=True, stop=True)
                            ubrep = scn.tile([128, L], BF16, name="ubrep",
                                             tag="ubrep", bufs=2)
                            nc.scalar.copy(ubrep[:], pru[:])
                            bk = scn.tile([128, L], BF16, name="bk", tag="bk", bufs=2)
                            nc.vector.tensor_tensor(bk[:], ubrep[:], Brep[:, bsl],
                                                    OP.mult)
                            hk = scn.tile([128, L], BF16, name="hk", tag="dtf16", bufs=2)
                            nc.vector.tensor_tensor_scan(hk[:], dA[:], bk[:], 0.0,
                                                         OP.mult, OP.add)
                            yck = scn.tile([128, L], BF16, name="yck", tag="ubrep",
                                           bufs=2)
                            nc.gpsimd.tensor_tensor(yck[:], hk[:], Crep[:, bsl],
                                                    OP.mult)
                            # pool: accumulate sum_n C*h into e-rows
                            for cc in range(2):
                                psl = slice(cc * 512, (cc + 1) * 512)
                                nc.tensor.matmul(pp[0:rows_out, psl],
                                                 poolm[:, k * 128:k * 128 + rows_out],
                                                 yck[:, cc * 512:(cc + 1) * 512],
                                                 start=(k == ks[0]), stop=(k == ks[-1]))
                        nc.scalar.copy(y_fm[g][0:rows_out, bsl], pp[0:rows_out, :])
                if _DEBUG and li == 0:
                    for ti, (eo, el) in enumerate(_etiles()):
                        ydd = st2.tile([el, T], FP32, name="ydd", tag="xcd")
                        nc.vector.tensor_copy(ydd[:], y_fm[ti][:el, :])
                        nc.sync.dma_start(dbg["yssm"][eo:eo + el, :], ydd[:])

                # ---- D-term, z-gate ----
                for ti, (eo, el) in enumerate(_etiles()):
                    nc.vector.scalar_tensor_tensor(y_fm[ti][:el, :], xc[ti][:el, :],
                                                   w_D[0:el, ti:ti + 1], y_fm[ti][:el, :],
                                                   OP.mult, OP.add)
                    nc.scalar.activation(z_sb[ti][:el, :], z_sb[ti][:el, :], AF.Silu)
                    nc.vector.tensor_tensor(y_fm[ti][:el, :], y_fm[ti][:el, :],
                                            z_sb[ti][:el, :], OP.mult)

                # ---- out_proj partial + AllReduce (split halves, overlap) ----
                op_outs = []
                for b_ in range(B):
                    op_in = dramp.tile([DM, L], BF16, name="op_in", tag="op_in")
                    op_out = dramp.tile([DM, L], BF16, name="op_out", tag="op_out",
                                        addr_space="Shared", bufs=2 * NL)
                    op_outs.append(op_out)
                    for mt in range(DTILES):
                        for f in range(2):
                            fs = slice(b_ * L + f * 512, b_ * L + (f + 1) * 512)
                            pt = pmm([128, 512])
                            for ti, (eo, el) in enumerate(_etiles()):
                                nc.tensor.matmul(
                                    pt[:],
                                    w_out[0:el, ti * DM + mt * 128:ti * DM + (mt + 1) * 128],
                                    y_fm[ti][:el, fs], start=(ti == 0), stop=(ti == 1))
                            opf = st2.tile([128, 512], BF16, name="opf", tag="opf",
                                           bufs=2)
                            nc.scalar.copy(opf[:], pt[:])
                            nc.sync.dma_start(
                                op_in[mt * 128:(mt + 1) * 128,
                                      f * 512:(f + 1) * 512], opf[:])
                    nc.gpsimd.collective_compute("AllReduce", OP.add,
                                                 replica_groups=[list(range(NC))],
                                                 ins=[op_in[:]], outs=[op_out[:]])
                for j in range(DTILES):
                    for b_ in range(B):
                        hs_f = st2.tile([128, L], BF16, name="hs_f", tag="hs_f",
                                        bufs=2)
                        nc.sync.dma_start(hs_f[:],
                                          op_outs[b_][j * 128:(j + 1) * 128, :])
                        nc.vector.tensor_tensor(
                            residual[j][:, b_ * L:(b_ + 1) * L],
                            residual[j][:, b_ * L:(b_ + 1) * L], hs_f[:], OP.add)

            # ================= final stage =================
            mixed = [actp.tile([128, T], BF16, name=f"mx{j}", tag=t)
                     for j, t in enumerate(
                         ["xp_pad0", "xp_pad1", "xc0", "xc1", "z_sb0", "z_sb1"])]

            def consume_mixed(f, sl6):
                fs = slice(f * 512, (f + 1) * 512)
                for j in range(DTILES):
                    nc.vector.tensor_copy(mixed[j][:, fs], sl6[j][:])

            ln_fm([nrmc[:, 2 * j:2 * j + 1] for j in range(DTILES)],
                  [nrmc[:, 2 * j + 1:2 * j + 2] for j in range(DTILES)],
                  consume_mixed)

            brow = actp.tile([1, T], BF16, name="brow", tag="stats16")
            nc.sync.dma_start(brow[:], bprob_d[:])

            wc1 = wpool.tile([128, 7 * GDM], BF16, name="wc1", tag="w_in_sb")
            nc.sync.dma_start(wc1[:], w_c1_d[:])
            bc1 = wpool.tile([GDM, 1], FP32, name="bc1", tag="w_cb_sb")
            nc.sync.dma_start(bc1[:], b_c1_d[:])
            wc2 = wpool.tile([GDM + 1, DM], BF16, name="wc2", tag="w_out_sb")
            nc.sync.dma_start(wc2[:], w_c2_d[:])

            h1 = actp.tile([GDM + 1, T], BF16, name="h1", tag="stats16b")
            nc.vector.memset(h1[GDM:GDM + 1, :], 1.0)
            for f in range(T // 512):
                fs = slice(f * 512, (f + 1) * 512)
                xfb = [st2.tile([128, 512], BF16, name=f"xfb{j}", tag=f"hlnf{j}")
                       for j in range(DTILES)]
                for c4 in range(4):
                    c = f * 4 + c4
                    x_tm_c = st2.tile([128, DM], FP32, name="x_tm_c3", tag="x_tm_c")
                    nc.sync.dma_start(x_tm_c[:], x_d[c * Q:(c + 1) * Q, :])
                    for j in range(DTILES):
                        ptt = pmm([128, 128])
                        nc.tensor.transpose(ptt[:], x_tm_c[:, j * 128:(j + 1) * 128],
                                            ident32[:])
                        nc.scalar.copy(xfb[j][:, c4 * 128:(c4 + 1) * 128], ptt[:])
                pt = pmm([GDM, 512])
                for kt in range(DTILES):
                    nc.tensor.matmul(pt[:], wc1[:, kt * GDM:(kt + 1) * GDM],
                                     xfb[kt][:], start=(kt == 0), stop=False)
                nc.tensor.matmul(pt[:], wc1[0:1, 6 * GDM:7 * GDM], brow[:, fs],
                                 start=False, stop=True)
                nc.scalar.activation(h1[0:GDM, fs], pt[:], AF.Silu, bias=bc1[:, 0:1])

            g_in = dramp.tile([T, DM], BF16, name="g_in", tag="g_in")
            g_out = dramp.tile([T, DM], BF16, name="g_out", tag="g_out",
                               addr_space="Shared")
            for c in range(NCH):
                h2sb = st2.tile([128, DM], BF16, name="h2sb", tag="h2sb")
                for fs2 in range(2):
                    pt = pmm([128, 384])
                    nc.tensor.matmul(pt[:], h1[:, c * Q:(c + 1) * Q],
                                     wc2[:, fs2 * 384:(fs2 + 1) * 384],
                                     start=True, stop=True)
                    nc.scalar.copy(h2sb[:, fs2 * 384:(fs2 + 1) * 384], pt[:])
                nc.sync.dma_start(g_in[c * Q:(c + 1) * Q, :], h2sb[:])
            nc.gpsimd.collective_compute("AllReduce", OP.add,
                                         replica_groups=[list(range(NC))],
                                         ins=[g_in[:]], outs=[g_out[:]])

            n16 = actp.tile([1, DM], BF16, name="n16", tag="n16")
            n16b = actp.tile([1, DM], BF16, name="n16b", tag="n16b")
            nr32 = st2.tile([1, DM], FP32, name="nr32", tag="h2sb")
            nc.sync.dma_start(nr32[:], nrm_d[2:3, :])
            nc.vector.tensor_copy(n16[:], nr32[:])
            nr32b = st2.tile([1, DM], FP32, name="nr32b", tag="h2sb")
            nc.sync.dma_start(nr32b[:], nrm_d[3:4, :])
            nc.vector.tensor_copy(n16b[:], nr32b[:])
            nfw_rep = actp.tile([128, DM], BF16, name="nfw_rep", tag="nfw_rep")
            nfb_rep = actp.tile([128, DM], BF16, name="nfb_rep", tag="nfb_rep")
            for fs2 in range(2):
                rp = pmm([128, 384])
                nc.tensor.matmul(rp[:], onesrow16[:1, :128],
                                 n16[0:1, fs2 * 384:(fs2 + 1) * 384], start=True, stop=True)
                nc.scalar.copy(nfw_rep[:, fs2 * 384:(fs2 + 1) * 384], rp[:])
                rp2 = pmm([128, 384])
                nc.tensor.matmul(rp2[:], onesrow16[:1, :128],
                                 n16b[0:1, fs2 * 384:(fs2 + 1) * 384], start=True, stop=True)
                nc.scalar.copy(nfb_rep[:, fs2 * 384:(fs2 + 1) * 384], rp2[:])

            for c in range(NCH):
                mixed_tm = st2.tile([128, DM], BF16, name="mixed_tm", tag="mixed_tm")
                for j in range(DTILES):
                    ptt = pmm([128, 128], BF16)
                    nc.tensor.transpose(ptt[:], mixed[j][:, c * Q:(c + 1) * Q], ident16[:])
                    nc.scalar.copy(mixed_tm[:, j * 128:(j + 1) * 128], ptt[:])
                xt = st2.tile([128, DM], FP32, name="xt", tag="x_tm_c")
                nc.sync.dma_start(xt[:], x_d[c * Q:(c + 1) * Q, :])
                gt16 = st2.tile([128, DM], BF16, name="gt16", tag="gt16", bufs=2)
                nc.sync.dma_start(gt16[:], g_out[c * Q:(c + 1) * Q, :])
                gt = st2.tile([128, DM], FP32, name="gt", tag="cacc", bufs=1)
                nc.scalar.activation(gt[:], gt16[:], AF.Sigmoid)
                nc.sync.dma_start(gate_d[c * Q:(c + 1) * Q, :], gt[:])
                ot = st2.tile([128, DM], FP32, name="ot", tag="ot", bufs=1)
                nc.vector.tensor_tensor(ot[:], mixed_tm[:], xt[:], OP.subtract)
                nc.vector.tensor_tensor(ot[:], ot[:], gt[:], OP.mult)
                nc.vector.tensor_tensor(ot[:], ot[:], xt[:], OP.add)
                st = st2.tile([128, 1], FP32, name="st", tag="st")
                nc.vector.tensor_reduce(st[:], ot[:], axis=AX.X, op=OP.add)
                nc.scalar.activation(st[:], st[:], AF.Copy, scale=1.0 / DM)
                nc.vector.tensor_scalar(ot[:], ot[:], st[:, 0:1], None, OP.subtract)
                sq2 = st2.tile([128, DM], FP32, name="sq2", tag="h2sb")
                nc.vector.tensor_tensor(sq2[:], ot[:], ot[:], OP.mult)
                v2 = st2.tile([128, 1], FP32, name="v2", tag="v2")
                nc.vector.tensor_reduce(v2[:], sq2[:], axis=AX.X, op=OP.add)
                nc.scalar.activation(v2[:], v2[:], AF.Ln, bias=eps_ap[:], scale=1.0 / DM)
                nc.scalar.activation(v2[:], v2[:], AF.Exp, scale=-0.5)
                nc.vector.tensor_scalar(ot[:], ot[:], v2[:, 0:1], None, OP.mult)
                nc.vector.tensor_tensor(ot[:], ot[:], nfw_rep[:], OP.mult)
                nc.vector.tensor_tensor(ot[:], ot[:], nfb_rep[:], OP.add)
                nc.sync.dma_start(out_d[c * Q:(c + 1) * Q, :], ot[:])

    nc.compile()
    return nc


def _pack_fm(arr, pad_to=128):
    arr = np.asarray(arr)
    if arr.ndim == 1:
        arr = arr[:, None]
    F, W = arr.shape
    nblk = (F + pad_to - 1) // pad_to
    outp = np.zeros((pad_to, nblk * W), dtype=arr.dtype)
    for b_ in range(nblk):
        blk = arr[b_ * pad_to:(b_ + 1) * pad_to]
        outp[:blk.shape[0], b_ * W:(b_ + 1) * W] = blk
    return outp


def _prep_inputs(inputs):
    f32 = np.float32
    x = np.ascontiguousarray(np.asarray(inputs["x"], f32).reshape(T, DM))
    bprob = np.ascontiguousarray(np.asarray(inputs["boundary_prob"], f32).reshape(1, T))
    A = -np.exp(np.asarray(inputs["A_log"], f32))
    a_scales = A[0, 0, :]

    r = np.arange(128)
    # REPA[e', 128k + r] = (e' == 8k + r%8) for k<16 (contraction over e' in [0,128))
    repa = np.zeros((128, 16 * 128), f32)
    for k in range(16):
        repa[:, 128 * k:128 * (k + 1)] = (np.arange(128)[:, None] ==
                                          (8 * k + (r % 8))[None, :])
    repb = np.zeros((64, 8 * 128), f32)
    for kk in range(8):
        repb[:, 128 * kk:128 * (kk + 1)] = (np.arange(64)[:, None] ==
                                            (8 * kk + (r % 8))[None, :])
    # POOL[r, 128k + j] = (j == 8*(k%16) + r%8)
    poolm = np.zeros((128, NKT * 128), f32)
    for k in range(NKT):
        poolm[:, 128 * k:128 * (k + 1)] = ((8 * (k % 16) + (r % 8))[:, None] ==
                                           np.arange(128)[None, :])
    # REPN[n, r] = (n == r//8)   (Brep/Crep: B16 row n -> rows 8n..8n+7)
    repn = (np.arange(16)[:, None] == (r // 8)[None, :]).astype(f32)
    acol = a_scales[(r // 8) % 16].astype(f32)[:, None]

    maps = []
    for c in range(NC):
        sl = slice(c * ELOC, (c + 1) * ELOC)
        w_in = np.stack([_pack_fm(
            np.concatenate([np.asarray(inputs["in_proj_w"][i])[sl],
                            np.asarray(inputs["in_proj_w"][i])[E + c * ELOC:E + (c + 1) * ELOC]],
                           axis=0).T.astype(f32))
            for i in range(NL)])
        w_xp = np.stack([_pack_fm(np.asarray(inputs["x_proj_w"][i], f32)[:, sl].T)
                         for i in range(NL)])
        w_dt = np.stack([np.asarray(inputs["dt_proj_w"][i], f32)[sl].T
                         for i in range(NL)])
        dtb = np.stack([_pack_fm(np.asarray(inputs["dt_proj_b"][i], f32)[sl])
                        for i in range(NL)])
        w_out = np.stack([_pack_fm(np.asarray(inputs["out_proj_w"][i], f32)[:, sl].T)
                          for i in range(NL)])
        lnp = np.stack([_pack_fm(np.stack([np.asarray(inputs["ln_w"][i], f32),
                                           np.asarray(inputs["ln_b"][i], f32)], axis=1))
                        for i in range(NL)])
        gsl = slice(c * GDM, (c + 1) * GDM)
        cw1 = np.asarray(inputs["ctrl_w1"], f32)
        w_c1 = np.concatenate([_pack_fm(cw1[gsl, :DM].T),
                               _pack_fm(cw1[gsl, DM:DM + 1].T)], axis=1)
        w_c2 = np.concatenate([np.asarray(inputs["ctrl_w2"], f32)[:, gsl].T,
                               (np.asarray(inputs["ctrl_b2"], f32) / NC)[None, :]], axis=0)
        nrm = np.stack([np.asarray(inputs["normf_w"], f32), np.asarray(inputs["normf_b"], f32),
                        np.asarray(inputs["out_ln_w"], f32), np.asarray(inputs["out_ln_b"], f32)])
        nrmc = _pack_fm(np.stack([np.asarray(inputs["normf_w"], f32),
                                  np.asarray(inputs["normf_b"], f32)], axis=1))
        maps.append({
            "x": x, "bprob": bprob, "w_in": w_in,
            "conv_w": np.stack([_pack_fm(np.asarray(inputs["conv_w"][i], f32)[sl])
                                for i in range(NL)]),
            "conv_b": np.stack([_pack_fm(np.asarray(inputs["conv_b"][i], f32)[sl])
                                for i in range(NL)]),
            "w_xp": w_xp, "w_dt": w_dt, "dtb": dtb, "w_out": w_out, "lnp": lnp,
            "ssmd": np.stack([_pack_fm(np.asarray(inputs["ssm_D"][i], f32)[sl])
                              for i in range(NL)]),
            "w_c1": w_c1,
            "b_c1": np.asarray(inputs["ctrl_b1"], f32)[gsl][:, None],
            "w_c2": w_c2, "nrm": nrm, "nrmc": nrmc,
            "repa": repa, "repb": repb, "poolm": poolm, "repn": repn,
            "acol": acol,
        })
    return maps


def _cast_bf16(maps):
    import ml_dtypes
    for m in maps:
        for k in ("w_in", "w_xp", "w_dt", "w_out", "w_c1", "w_c2", "bprob",
                  "repa", "repb", "poolm", "repn"):
            m[k] = np.asarray(m[k], dtype=ml_dtypes.bfloat16)
    return maps


def kernel(**inputs):
    maps = _prep_inputs(inputs)
    A = -np.exp(np.asarray(inputs["A_log"], np.float32))
    a_scales = A[0, 0, :]
    for i in range(NL):
        assert np.allclose(A[i], np.broadcast_to(a_scales, (E, N)), rtol=1e-5, atol=1e-6), \
            "kernel assumes channel-independent A"
    if "nc" not in _CACHE:
        _CACHE["nc"] = _build()
    nc = _CACHE["nc"]
    _cast_bf16(maps)
    res = run_bass_kernel_spmd(nc, maps, list(range(NC)))
    kernel._res = res
    r0 = res.results[0]
    out = np.asarray(r0["out"], np.float32).reshape(B, L, DM)
    gate = np.asarray(r0["gate"], np.float32).reshape(B, L, DM)
    return out, gate



# revision 17
# speedup vs baseline: 1.6469x; 1.0231x over previous
"""Trainium2 Bass kernel for nn_BoundaryControlledMixer (4-layer Mamba stack +
boundary-controlled gate), tensor-parallel over d_inner across 8 NeuronCores.

Per core (owns E_loc = 192 of E = 1536 channels, full batch/sequence):
  - Activations flow feature-major [feat, token] so projections chain on the
    PE without transposes (matmul contracts the partition dim).
  - Selective scan: DVE tensor_tensor_scan (state = dA*state + b, fp32 state)
    over 24 row-tiles of the (n, e) grid: tile k holds rows r = n*8 + e_l,
    e = 8k + e_l, n = r//8, scanned along the full token axis [128, T].
    Batch reset via dA[:, L] = 0.  Inputs built by PE 0/1-replication
    (dt -> dtrep -> exp = dA) and log-doubling DMA (u = dt*xc -> ubrep),
    b = ubrep * Brep.  y = sum_n C*h via a 0/1 pooling matmul.
"""

import numpy as np

import concourse.bacc as bacc
import concourse.bass as bass
import concourse.mybir as mybir
import concourse.tile as tile
from concourse import masks
from concourse.bass_utils import run_bass_kernel_spmd

FP32 = mybir.dt.float32
BF16 = mybir.dt.bfloat16
AF = mybir.ActivationFunctionType
OP = mybir.AluOpType
AX = mybir.AxisListType

B, L, DM, NL = 2, 1024, 768, 4
E, N, K, R = 2 * DM, 16, 4, DM // 16
NC = 8
ELOC = E // NC            # 192
T = B * L                 # 2048
Q = 128
NCH = T // Q              # 16
EPS = 1e-5
DTILES = DM // 128        # 6
GDM = DM // NC            # 96
LPD = L + 2 * K           # padded per-batch xp row
NKT = N * ELOC // 128     # 24 scan tiles
RD = R + 1                # 49

_CACHE = {}
_DEBUG = False


def _etiles():
    return [(0, 128), (128, 64)]


def _build():
    nc = bacc.Bacc("TRN2", target_bir_lowering=False, debug=False)

    x_d = nc.dram_tensor("x", [T, DM], FP32, kind="ExternalInput")
    bprob_d = nc.dram_tensor("bprob", [1, T], BF16, kind="ExternalInput")
    w_in_d = nc.dram_tensor("w_in", [NL, 128, 6 * 2 * ELOC], BF16, kind="ExternalInput")
    conv_w_d = nc.dram_tensor("conv_w", [NL, 128, 2 * K], FP32, kind="ExternalInput")
    conv_b_d = nc.dram_tensor("conv_b", [NL, 128, 2], FP32, kind="ExternalInput")
    w_xp_d = nc.dram_tensor("w_xp", [NL, 128, 2 * (R + 2 * N)], BF16, kind="ExternalInput")
    w_dt_d = nc.dram_tensor("w_dt", [NL, R, ELOC], BF16, kind="ExternalInput")
    dtb_d = nc.dram_tensor("dtb", [NL, 128, 2], FP32, kind="ExternalInput")
    w_out_d = nc.dram_tensor("w_out", [NL, 128, 2 * DM], BF16, kind="ExternalInput")
    ln_d = nc.dram_tensor("lnp", [NL, 128, 12], FP32, kind="ExternalInput")
    ssmd_d = nc.dram_tensor("ssmd", [NL, 128, 2], FP32, kind="ExternalInput")
    w_c1_d = nc.dram_tensor("w_c1", [128, 7 * GDM], BF16, kind="ExternalInput")
    b_c1_d = nc.dram_tensor("b_c1", [GDM, 1], FP32, kind="ExternalInput")
    w_c2_d = nc.dram_tensor("w_c2", [GDM + 1, DM], BF16, kind="ExternalInput")
    nrm_d = nc.dram_tensor("nrm", [4, DM], FP32, kind="ExternalInput")
    nrmc_d = nc.dram_tensor("nrmc", [128, 12], FP32, kind="ExternalInput")
    repa_d = nc.dram_tensor("repa", [128, 16 * 128], BF16, kind="ExternalInput")
    repb_d = nc.dram_tensor("repb", [64, 8 * 128], BF16, kind="ExternalInput")
    pool_d = nc.dram_tensor("poolm", [128, NKT * 128], BF16, kind="ExternalInput")
    repn_d = nc.dram_tensor("repn", [16, 128], BF16, kind="ExternalInput")
    acol_d = nc.dram_tensor("acol", [128, 1], FP32, kind="ExternalInput")

    out_d = nc.dram_tensor("out", [T, DM], FP32, kind="ExternalOutput")
    gate_d = nc.dram_tensor("gate", [T, DM], FP32, kind="ExternalOutput")
    dbg = {}
    if _DEBUG:
        dbg["dt"] = nc.dram_tensor("dbg_dt", [ELOC, T], FP32, kind="ExternalOutput")
        dbg["yssm"] = nc.dram_tensor("dbg_yssm", [ELOC, T], FP32, kind="ExternalOutput")
        dbg["xc"] = nc.dram_tensor("dbg_xc", [ELOC, T], FP32, kind="ExternalOutput")
        dbg["hln"] = nc.dram_tensor("dbg_hln", [DM, T], FP32, kind="ExternalOutput")

    with tile.TileContext(nc) as tc:
        with tc.tile_pool(name="const", bufs=1) as constp, \
             tc.tile_pool(name="persist", bufs=1) as pers, \
             tc.tile_pool(name="wts", bufs=1) as wpool, \
             tc.tile_pool(name="act", bufs=1) as actp, \
             tc.tile_pool(name="st2", bufs=1) as st2, \
             tc.tile_pool(name="scn", bufs=1) as scn, \
             tc.tile_pool(name="ps_rep", bufs=1, space="PSUM") as ps_rep, \
             tc.tile_pool(name="ps_pool", bufs=1, space="PSUM") as ps_pool, \
             tc.tile_pool(name="ps_mm", bufs=2, space="PSUM") as ps_mm, \
             tc.tile_pool(name="dram", bufs=2, space="DRAM") as dramp:

            def pmm(shape, dt=FP32):
                return ps_mm.tile(shape, dt, name="pmm", tag="pmm")

            # ---------- constants ----------
            ident32 = constp.tile([128, 128], FP32)
            masks.make_identity(nc, ident32[:])
            ident16 = constp.tile([128, 128], BF16)
            masks.make_identity(nc, ident16[:])
            onesrow16 = constp.tile([1, 512], BF16)
            nc.gpsimd.memset(onesrow16[:], 1.0)
            halfcol32 = constp.tile([128, 1], FP32)
            nc.gpsimd.memset(halfcol32[:], 0.5)
            halfcol16 = constp.tile([128, 1], BF16)
            nc.gpsimd.memset(halfcol16[:], 0.5)
            eps_ap = constp.tile([128, 1], FP32)
            nc.gpsimd.memset(eps_ap[:], EPS)
            nrmc = constp.tile([128, 12], FP32)
            nc.sync.dma_start(nrmc[:], nrmc_d[:])
            repa = constp.tile([128, 16 * 128], BF16)
            nc.sync.dma_start(repa[:], repa_d[:])
            repb = constp.tile([64, 8 * 128], BF16)
            nc.sync.dma_start(repb[:], repb_d[:])
            poolm = constp.tile([128, NKT * 128], BF16)
            nc.sync.dma_start(poolm[:], pool_d[:])
            repn = constp.tile([16, 128], BF16)
            nc.sync.dma_start(repn[:], repn_d[:])
            acol = constp.tile([128, 1], FP32)
            nc.sync.dma_start(acol[:], acol_d[:])

            # ---------- x -> feature-major fp32 residual ----------
            residual = [pers.tile([128, T], FP32, name=f"res{j}") for j in range(DTILES)]
            for c in range(NCH):
                x_tm_c = st2.tile([128, DM], FP32, name="x_tm_c", tag="x_tm_c")
                nc.sync.dma_start(x_tm_c[:], x_d[c * Q:(c + 1) * Q, :])
                for j in range(DTILES):
                    pt = pmm([128, 128])
                    nc.tensor.transpose(pt[:], x_tm_c[:, j * 128:(j + 1) * 128], ident32[:])
                    nc.scalar.copy(residual[j][:, c * Q:(c + 1) * Q], pt[:])

            # ---------- fused feature-major LayerNorm ----------
            def ln_fm(lnw_aps, lnb_aps, consume, dbg_dst=None):
                stats16 = actp.tile([1, T], BF16, name="stats16", tag="stats16")
                stats16b = actp.tile([1, T], BF16, name="stats16b", tag="stats16b")

                for f in range(T // 512):
                    fs = slice(f * 512, (f + 1) * 512)
                    sp1 = pmm([1, 512])
                    sp2 = pmm([1, 512])
                    for j in range(DTILES):
                        nc.tensor.matmul(sp1[:], halfcol32, residual[j][:, fs],
                                         start=(j == 0), stop=(j == DTILES - 1))
                    nc.scalar.activation(stats16[0:1, fs], sp1[:], AF.Copy, scale=2.0 / DM)
                    for j in range(DTILES):
                        sqj = st2.tile([128, 512], BF16, name="ln_sqj", tag="ln_sqj")
                        nc.vector.tensor_tensor(sqj[:], residual[j][:, fs],
                                                residual[j][:, fs], OP.mult)
                        nc.tensor.matmul(sp2[:], halfcol16[:], sqj[:],
                                         start=(j == 0), stop=(j == DTILES - 1))
                    nc.scalar.activation(stats16b[0:1, fs], sp2[:], AF.Copy, scale=2.0 / DM)

                for f in range(T // 512):
                    fs = slice(f * 512, (f + 1) * 512)
                    rp = pmm([128, 512])
                    nc.tensor.matmul(rp[:], onesrow16[:1, :128], stats16[0:1, fs],
                                     start=True, stop=True)
                    meanr = st2.tile([128, 512], FP32, name="ln_meanr", tag="ln_meanr")
                    nc.scalar.copy(meanr[:], rp[:])
                    rp2 = pmm([128, 512])
                    nc.tensor.matmul(rp2[:], onesrow16[:1, :128], stats16b[0:1, fs],
                                     start=True, stop=True)
                    invr = st2.tile([128, 512], FP32, name="ln_invr", tag="ln_invr")
                    nc.scalar.copy(invr[:], rp2[:])
                    # var = E[x^2] - mean^2 ; inv = exp(-0.5*ln(var+eps))
                    c2r = st2.tile([128, 512], FP32, name="ln_c2r", tag="ln_c2r")
                    nc.vector.tensor_tensor(c2r[:], meanr[:], meanr[:], OP.mult)
                    nc.vector.tensor_tensor(invr[:], invr[:], c2r[:], OP.subtract)
                    nc.scalar.activation(invr[:], invr[:], AF.Ln, bias=eps_ap[:])
                    nc.scalar.activation(invr[:], invr[:], AF.Exp, scale=-0.5)
                    nc.vector.tensor_tensor(c2r[:], meanr[:], invr[:], OP.mult)
                    slices = []
                    for j in range(DTILES):
                        tmp = st2.tile([128, 512], BF16, name="ln_tmp", tag="ln_tmp", bufs=2)
                        nc.vector.tensor_tensor(tmp[:], residual[j][:, fs], invr[:], OP.mult)
                        nc.vector.tensor_tensor(tmp[:], tmp[:], c2r[:], OP.subtract)
                        hlnf = st2.tile([128, 512], BF16, name="hlnf", tag=f"hlnf{j}")
                        nc.scalar.activation(hlnf[:], tmp[:], AF.Identity,
                                             scale=lnw_aps[j], bias=lnb_aps[j])
                        slices.append(hlnf)
                        if dbg_dst is not None:
                            hld = st2.tile([128, 512], FP32, name="hld", tag="hld")
                            nc.vector.tensor_copy(hld[:], hlnf[:])
                            nc.sync.dma_start(dbg_dst[j * 128:(j + 1) * 128, fs], hld[:])
                    consume(f, slices)

            # ================= layers =================
            for li in range(NL):
                w_in = wpool.tile([128, 6 * 2 * ELOC], BF16, name="w_in_sb", tag="w_in_sb")
                nc.sync.dma_start(w_in[:], w_in_d[li])
                w_cw = wpool.tile([128, 2 * K], FP32, name="w_cw_sb", tag="w_cw_sb")
                nc.sync.dma_start(w_cw[:], conv_w_d[li])
                w_cb = wpool.tile([128, 2], FP32, name="w_cb_sb", tag="w_cb_sb")
                nc.sync.dma_start(w_cb[:], conv_b_d[li])
                w_xp = wpool.tile([128, 2 * (R + 2 * N)], BF16, name="w_xp_sb", tag="w_xp_sb")
                nc.sync.dma_start(w_xp[:], w_xp_d[li])
                w_dt = wpool.tile([R, ELOC], BF16, name="w_dt_sb", tag="w_dt_sb")
                nc.sync.dma_start(w_dt[:], w_dt_d[li])
                dtb = wpool.tile([128, 2], FP32, name="dtb_sb", tag="dtb_sb")
                nc.sync.dma_start(dtb[:], dtb_d[li])
                w_out = wpool.tile([128, 2 * DM], BF16, name="w_out_sb", tag="w_out_sb")
                nc.sync.dma_start(w_out[:], w_out_d[li])
                w_ln = wpool.tile([128, 12], FP32, name="w_ln_sb", tag="w_ln_sb")
                nc.sync.dma_start(w_ln[:], ln_d[li])
                w_D = wpool.tile([128, 2], FP32, name="w_D_sb", tag="w_D_sb")
                nc.sync.dma_start(w_D[:], ssmd_d[li])

                # ---- LN fused with in_proj ----
                xp_t = [actp.tile([128, B * LPD], BF16, name="xp_pad0", tag="xp_pad0"),
                        actp.tile([64, B * LPD], BF16, name="xp_pad1", tag="xp_pad1")]
                z_sb = [actp.tile([128, T], BF16, name="z_sb0", tag="z_sb0"),
                        actp.tile([64, T], BF16, name="z_sb1", tag="z_sb1")]
                for ti in range(2):
                    nc.vector.memset(xp_t[ti][:, 0:K], 0.0)
                    nc.vector.memset(xp_t[ti][:, LPD:LPD + K], 0.0)

                def padcol(fs, fl):
                    b_ = fs // L
                    off = b_ * LPD + K + (fs - b_ * L)
                    return slice(off, off + fl)

                def consume_inproj(f, sl6):
                    fs = f * 512
                    for mt in range(3):
                        pt = pmm([128, 512])
                        for kt in range(DTILES):
                            nc.tensor.matmul(
                                pt[:], w_in[:, kt * 384 + mt * 128:kt * 384 + (mt + 1) * 128],
                                sl6[kt][:], start=(kt == 0), stop=(kt == DTILES - 1))
                        if mt == 0:
                            nc.scalar.copy(xp_t[0][:, padcol(fs, 512)], pt[:])
                        elif mt == 1:
                            nc.scalar.copy(xp_t[1][:, padcol(fs, 512)], pt[0:64, :])
                            nc.scalar.copy(z_sb[0][0:64, fs:fs + 512], pt[64:128, :])
                        else:
                            nc.scalar.copy(z_sb[0][64:128, fs:fs + 512], pt[0:64, :])
                            nc.scalar.copy(z_sb[1][0:64, fs:fs + 512], pt[64:128, :])

                ln_fm([w_ln[:, 2 * j:2 * j + 1] for j in range(DTILES)],
                      [w_ln[:, 2 * j + 1:2 * j + 2] for j in range(DTILES)],
                      consume_inproj,
                      dbg_dst=dbg["hln"] if (_DEBUG and li == 0) else None)

                # ---- conv + silu ----
                xc = [actp.tile([128, T], BF16, name="xc0", tag="xc0"),
                      actp.tile([64, T], BF16, name="xc1", tag="xc1")]
                for ti, (eo, el) in enumerate(_etiles()):
                    for b_ in range(B):
                        acc = st2.tile([el, L], FP32, name="cacc", tag="cacc", bufs=2)
                        cb = b_ * LPD + K
                        nc.vector.tensor_scalar(acc[:], xp_t[ti][:el, cb - 3:cb - 3 + L],
                                                w_cw[0:el, ti * K:ti * K + 1], None, OP.mult)
                        for j in range(1, K):
                            nc.vector.scalar_tensor_tensor(
                                acc[:], xp_t[ti][:el, cb - 3 + j:cb - 3 + j + L],
                                w_cw[0:el, ti * K + j:ti * K + j + 1],
                                acc[:], OP.mult, OP.add)
                        nc.scalar.activation(xc[ti][:el, b_ * L:(b_ + 1) * L], acc[:],
                                             AF.Silu, bias=w_cb[0:el, ti:ti + 1])
                if _DEBUG and li == 0:
                    for ti, (eo, el) in enumerate(_etiles()):
                        xcd = st2.tile([el, T], FP32, name="xcd", tag="xcd")
                        nc.vector.tensor_copy(xcd[:], xc[ti][:el, :])
                        nc.sync.dma_start(dbg["xc"][eo:eo + el, :], xcd[:])

                # ---- x_proj partial + AllReduce (bf16 wire, Shared out) ----
                dbl_in = dramp.tile([R + 2 * N, T], BF16, name="dbl_in", tag="dbl_in")
                dbl_out = dramp.tile([R + 2 * N, T], BF16, name="dbl_out",
                                     tag="dbl_out", addr_space="Shared", bufs=NL)
                for f in range(T // 512):
                    fs = slice(f * 512, (f + 1) * 512)
                    pt = pmm([80, 512])
                    for ti, (eo, el) in enumerate(_etiles()):
                        nc.tensor.matmul(pt[:], w_xp[0:el, ti * 80:(ti + 1) * 80],
                                         xc[ti][:el, fs], start=(ti == 0), stop=(ti == 1))
                    dblf = st2.tile([80, 512], BF16, name="dblf", tag="dblf")
                    nc.scalar.copy(dblf[:], pt[:])
                    nc.sync.dma_start(dbl_in[:, fs], dblf[:])
                nc.gpsimd.collective_compute("AllReduce", OP.add,
                                             replica_groups=[list(range(NC))],
                                             ins=[dbl_in[:]], outs=[dbl_out[:]])

                # ---- dt path: softplus(w_dt @ dbl + b) e-major ----
                dtf16 = scn.tile([R, T], BF16, name="dtf16", tag="dtf16", bufs=2)
                nc.sync.dma_start(dtf16[:], dbl_out[0:R, :])

                dt16 = [scn.tile([128, T], BF16, name="dt16_0", tag="dt16_0"),
                        scn.tile([64, T], BF16, name="dt16_1", tag="dt16_1")]
                for ti, (eo, el) in enumerate(_etiles()):
                    for h_ in range(4):
                        csl = slice(h_ * 512, (h_ + 1) * 512)
                        ptd = ps_rep.tile([128, 512], FP32, name="ptd", tag="rep",
                                          bufs=4)
                        nc.tensor.matmul(ptd[0:el, :],
                                         w_dt[:, eo:eo + el], dtf16[:, csl],
                                         start=True, stop=True)
                        e1 = scn.tile([el, 512], FP32, name="e1", tag="dstage")
                        nc.scalar.activation(e1[:], ptd[0:el, :], AF.Exp,
                                             bias=dtb[0:el, ti:ti + 1])
                        nc.scalar.activation(dt16[ti][:el, csl], e1[:], AF.Ln,
                                             bias=1.0)
                if _DEBUG and li == 0:
                    dtd = st2.tile([128, T], FP32, name="dtd", tag="dtd")
                    nc.vector.tensor_copy(dtd[:], dt16[0][:])
                    nc.sync.dma_start(dbg["dt"][0:128, :], dtd[:])
                    dtd2 = st2.tile([64, T], FP32, name="dtd2", tag="dtd")
                    nc.vector.tensor_copy(dtd2[:], dt16[1][:])
                    nc.sync.dma_start(dbg["dt"][128:192, :], dtd2[:])

                # ---- u = dt * xc ; B/C replication ----
                u16 = [scn.tile([128, T], BF16, name="u16_0", tag="u16_0"),
                       scn.tile([64, T], BF16, name="u16_1", tag="u16_1")]
                for ti, (eo, el) in enumerate(_etiles()):
                    nc.vector.tensor_tensor(u16[ti][:el, :], dt16[ti][:el, :],
                                            xc[ti][:el, :], OP.mult)

                Brep = scn.tile([128, T], BF16, name="Brep", tag="Brep")
                Crep = scn.tile([128, T], BF16, name="Crep", tag="Crep")
                for si, dst in ((0, Brep), (1, Crep)):
                    src16 = scn.tile([16, T], BF16, name=f"bc16_{si}", tag="dtf16", bufs=2)
                    nc.sync.dma_start(src16[:], dbl_out[R + si * N:R + (si + 1) * N, :])
                    for h_ in range(4):
                        csl = slice(h_ * 512, (h_ + 1) * 512)
                        prr = ps_rep.tile([128, 512], FP32, name="prr", tag="rep",
                                          bufs=4)
                        nc.tensor.matmul(prr[:], repn[:], src16[:, csl],
                                         start=True, stop=True)
                        nc.scalar.copy(dst[:, csl], prr[:])

                # ---- scan tiles ----
                # Per (group, batch): pool accumulator [128, L] (2 PSUM banks);
                # dt/u replication via PE matmul into [128, L] PSUM (2 banks,
                # bufs=2), consumed by scalar (exp -> dA, copy -> urep).
                y_fm = [actp.tile([128, T], BF16, name="yfm0", tag="xp_pad0"),
                        actp.tile([64, T], BF16, name="yfm1", tag="xp_pad1")]
                for g, (ks, el_in) in enumerate(((range(0, 16), 128),
                                                 (range(16, 24), 64))):
                    rows_out = 128 if g == 0 else 64
                    repm = repa if g == 0 else repb
                    for b_ in range(B):
                        bsl = slice(b_ * L, (b_ + 1) * L)
                        pp = ps_pool.tile([128, L], FP32, name="pp", tag="pp")
                        for k in ks:
                            kk = k - 16 * g
                            # u replication via PE first (feeds bk)
                            ubrep = scn.tile([128, L], BF16, name="ubrep",
                                             tag="ubrep", bufs=2)
                            for cc in range(2):
                                psl = slice(cc * 512, (cc + 1) * 512)
                                csl = slice(b_ * L + cc * 512, b_ * L + (cc + 1) * 512)
                                pru = ps_rep.tile([128, 512], FP32, name="pru",
                                                  tag="rep", bufs=4)
                                nc.tensor.matmul(pru[:],
                                                 repm[:, kk * 128:(kk + 1) * 128],
                                                 u16[g][:el_in, csl],
                                                 start=True, stop=True)
                                nc.scalar.copy(ubrep[:, psl], pru[:])
                            # dt replication -> dA = exp(a * dtrep)
                            dA = scn.tile([128, L], FP32, name="dA", tag="dA", bufs=2)
                            for cc in range(2):
                                psl = slice(cc * 512, (cc + 1) * 512)
                                csl = slice(b_ * L + cc * 512, b_ * L + (cc + 1) * 512)
                                prp = ps_rep.tile([128, 512], FP32, name="prp",
                                                  tag="rep", bufs=4)
                                nc.tensor.matmul(prp[:],
                                                 repm[:, kk * 128:(kk + 1) * 128],
                                                 dt16[g][:el_in, csl],
                                                 start=True, stop=True)
                                nc.scalar.activation(dA[:, psl], prp[:], AF.Exp,
                                                     scale=acol[:, 0:1])
                            bk = scn.tile([128, L], BF16, name="bk", tag="bk", bufs=2)
                            nc.vector.tensor_tensor(bk[:], ubrep[:], Brep[:, bsl],
                                                    OP.mult)
                            hk = scn.tile([128, L], BF16, name="hk", tag="dtf16", bufs=2)
                            nc.vector.tensor_tensor_scan(hk[:], dA[:], bk[:], 0.0,
                                                         OP.mult, OP.add)
                            yck = scn.tile([128, L], BF16, name="yck", tag="yck",
                                           bufs=2)
                            nc.gpsimd.tensor_tensor(yck[:], hk[:], Crep[:, bsl],
                                                    OP.mult)
                            # pool: accumulate sum_n C*h into e-rows
                            for cc in range(2):
                                psl = slice(cc * 512, (cc + 1) * 512)
                                nc.tensor.matmul(pp[0:rows_out, psl],
                                                 poolm[:, k * 128:k * 128 + rows_out],
                                                 yck[:, cc * 512:(cc + 1) * 512],
                                                 start=(k == ks[0]), stop=(k == ks[-1]))
                        nc.scalar.copy(y_fm[g][0:rows_out, bsl], pp[0:rows_out, :])
                if _DEBUG and li == 0:
                    for ti, (eo, el) in enumerate(_etiles()):
                        ydd = st2.tile([el, T], FP32, name="ydd", tag="xcd")
                        nc.vector.tensor_copy(ydd[:], y_fm[ti][:el, :])
                        nc.sync.dma_start(dbg["yssm"][eo:eo + el, :], ydd[:])

                # ---- D-term, z-gate ----
                for ti, (eo, el) in enumerate(_etiles()):
                    nc.vector.scalar_tensor_tensor(y_fm[ti][:el, :], xc[ti][:el, :],
                                                   w_D[0:el, ti:ti + 1], y_fm[ti][:el, :],
                                                   OP.mult, OP.add)
                    nc.scalar.activation(z_sb[ti][:el, :], z_sb[ti][:el, :], AF.Silu)
                    nc.vector.tensor_tensor(y_fm[ti][:el, :], y_fm[ti][:el, :],
                                            z_sb[ti][:el, :], OP.mult)

                # ---- out_proj partial + AllReduce (split halves, overlap) ----
                op_outs = []
                for b_ in range(B):
                    op_in = dramp.tile([DM, L], BF16, name="op_in", tag="op_in")
                    op_out = dramp.tile([DM, L], BF16, name="op_out", tag="op_out",
                                        addr_space="Shared", bufs=2 * NL)
                    op_outs.append(op_out)
                    for mt in range(DTILES):
                        for f in range(2):
                            fs = slice(b_ * L + f * 512, b_ * L + (f + 1) * 512)
                            pt = pmm([128, 512])
                            for ti, (eo, el) in enumerate(_etiles()):
                                nc.tensor.matmul(
                                    pt[:],
                                    w_out[0:el, ti * DM + mt * 128:ti * DM + (mt + 1) * 128],
                                    y_fm[ti][:el, fs], start=(ti == 0), stop=(ti == 1))
                            opf = st2.tile([128, 512], BF16, name="opf", tag="opf",
                                           bufs=2)
                            nc.scalar.copy(opf[:], pt[:])
                            nc.sync.dma_start(
                                op_in[mt * 128:(mt + 1) * 128,
                                      f * 512:(f + 1) * 512], opf[:])
                    nc.gpsimd.collective_compute("AllReduce", OP.add,
                                                 replica_groups=[list(range(NC))],
                                                 ins=[op_in[:]], outs=[op_out[:]])
                for j in range(DTILES):
                    for b_ in range(B):
                        hs_f = st2.tile([128, L], BF16, name="hs_f", tag="hs_f",
                                        bufs=2)
                        nc.sync.dma_start(hs_f[:],
                                          op_outs[b_][j * 128:(j + 1) * 128, :])
                        nc.vector.tensor_tensor(
                            residual[j][:, b_ * L:(b_ + 1) * L],
                            residual[j][:, b_ * L:(b_ + 1) * L], hs_f[:], OP.add)

            # ================= final stage =================
            mixed = [actp.tile([128, T], BF16, name=f"mx{j}", tag=t)
                     for j, t in enumerate(
                         ["xp_pad0", "xp_pad1", "xc0", "xc1", "z_sb0", "z_sb1"])]

            def consume_mixed(f, sl6):
                fs = slice(f * 512, (f + 1) * 512)
                for j in range(DTILES):
                    nc.vector.tensor_copy(mixed[j][:, fs], sl6[j][:])

            ln_fm([nrmc[:, 2 * j:2 * j + 1] for j in range(DTILES)],
                  [nrmc[:, 2 * j + 1:2 * j + 2] for j in range(DTILES)],
                  consume_mixed)

            brow = actp.tile([1, T], BF16, name="brow", tag="stats16")
            nc.sync.dma_start(brow[:], bprob_d[:])

            wc1 = wpool.tile([128, 7 * GDM], BF16, name="wc1", tag="w_in_sb")
            nc.sync.dma_start(wc1[:], w_c1_d[:])
            bc1 = wpool.tile([GDM, 1], FP32, name="bc1", tag="w_cb_sb")
            nc.sync.dma_start(bc1[:], b_c1_d[:])
            wc2 = wpool.tile([GDM + 1, DM], BF16, name="wc2", tag="w_out_sb")
            nc.sync.dma_start(wc2[:], w_c2_d[:])

            h1 = actp.tile([GDM + 1, T], BF16, name="h1", tag="stats16b")
            nc.vector.memset(h1[GDM:GDM + 1, :], 1.0)
            for f in range(T // 512):
                fs = slice(f * 512, (f + 1) * 512)
                xfb = [st2.tile([128, 512], BF16, name=f"xfb{j}", tag=f"hlnf{j}")
                       for j in range(DTILES)]
                for c4 in range(4):
                    c = f * 4 + c4
                    x_tm_c = st2.tile([128, DM], FP32, name="x_tm_c3", tag="x_tm_c")
                    nc.sync.dma_start(x_tm_c[:], x_d[c * Q:(c + 1) * Q, :])
                    for j in range(DTILES):
                        ptt = pmm([128, 128])
                        nc.tensor.transpose(ptt[:], x_tm_c[:, j * 128:(j + 1) * 128],
                                            ident32[:])
                        nc.scalar.copy(xfb[j][:, c4 * 128:(c4 + 1) * 128], ptt[:])
                pt = pmm([GDM, 512])
                for kt in range(DTILES):
                    nc.tensor.matmul(pt[:], wc1[:, kt * GDM:(kt + 1) * GDM],
                                     xfb[kt][:], start=(kt == 0), stop=False)
                nc.tensor.matmul(pt[:], wc1[0:1, 6 * GDM:7 * GDM], brow[:, fs],
                                 start=False, stop=True)
                nc.scalar.activation(h1[0:GDM, fs], pt[:], AF.Silu, bias=bc1[:, 0:1])

            g_in = dramp.tile([T, DM], BF16, name="g_in", tag="g_in")
            g_out = dramp.tile([T, DM], BF16, name="g_out", tag="g_out",
                               addr_space="Shared")
            for c in range(NCH):
                h2sb = st2.tile([128, DM], BF16, name="h2sb", tag="h2sb")
                for fs2 in range(2):
                    pt = pmm([128, 384])
                    nc.tensor.matmul(pt[:], h1[:, c * Q:(c + 1) * Q],
                                     wc2[:, fs2 * 384:(fs2 + 1) * 384],
                                     start=True, stop=True)
                    nc.scalar.copy(h2sb[:, fs2 * 384:(fs2 + 1) * 384], pt[:])
                nc.sync.dma_start(g_in[c * Q:(c + 1) * Q, :], h2sb[:])
            nc.gpsimd.collective_compute("AllReduce", OP.add,
                                         replica_groups=[list(range(NC))],
                                         ins=[g_in[:]], outs=[g_out[:]])

            n16 = actp.tile([1, DM], BF16, name="n16", tag="n16")
            n16b = actp.tile([1, DM], BF16, name="n16b", tag="n16b")
            nr32 = st2.tile([1, DM], FP32, name="nr32", tag="h2sb")
            nc.sync.dma_start(nr32[:], nrm_d[2:3, :])
            nc.vector.tensor_copy(n16[:], nr32[:])
            nr32b = st2.tile([1, DM], FP32, name="nr32b", tag="h2sb")
            nc.sync.dma_start(nr32b[:], nrm_d[3:4, :])
            nc.vector.tensor_copy(n16b[:], nr32b[:])
            nfw_rep = actp.tile([128, DM], BF16, name="nfw_rep", tag="nfw_rep")
            nfb_rep = actp.tile([128, DM], BF16, name="nfb_rep", tag="nfb_rep")
            for fs2 in range(2):
                rp = pmm([128, 384])
                nc.tensor.matmul(rp[:], onesrow16[:1, :128],
                                 n16[0:1, fs2 * 384:(fs2 + 1) * 384], start=True, stop=True)
                nc.scalar.copy(nfw_rep[:, fs2 * 384:(fs2 + 1) * 384], rp[:])
                rp2 = pmm([128, 384])
                nc.tensor.matmul(rp2[:], onesrow16[:1, :128],
                                 n16b[0:1, fs2 * 384:(fs2 + 1) * 384], start=True, stop=True)
                nc.scalar.copy(nfb_rep[:, fs2 * 384:(fs2 + 1) * 384], rp2[:])

            for c in range(NCH):
                mixed_tm = st2.tile([128, DM], BF16, name="mixed_tm", tag="mixed_tm")
                for j in range(DTILES):
                    ptt = pmm([128, 128], BF16)
                    nc.tensor.transpose(ptt[:], mixed[j][:, c * Q:(c + 1) * Q], ident16[:])
                    nc.scalar.copy(mixed_tm[:, j * 128:(j + 1) * 128], ptt[:])
                xt = st2.tile([128, DM], FP32, name="xt", tag="x_tm_c")
                nc.sync.dma_start(xt[:], x_d[c * Q:(c + 1) * Q, :])
                gt16 = st2.tile([128, DM], BF16, name="gt16", tag="gt16", bufs=2)
                nc.sync.dma_start(gt16[:], g_out[c * Q:(c + 1) * Q, :])
                gt = st2.tile([128, DM], FP32, name="gt", tag="cacc", bufs=2)
                nc.scalar.activation(gt[:], gt16[:], AF.Sigmoid)
                nc.sync.dma_start(gate_d[c * Q:(c + 1) * Q, :], gt[:])
                ot = st2.tile([128, DM], FP32, name="ot", tag="ot", bufs=1)
                nc.vector.tensor_tensor(ot[:], mixed_tm[:], xt[:], OP.subtract)
                nc.vector.tensor_tensor(ot[:], ot[:], gt[:], OP.mult)
                nc.vector.tensor_tensor(ot[:], ot[:], xt[:], OP.add)
                st = st2.tile([128, 1], FP32, name="st", tag="st")
                nc.vector.tensor_reduce(st[:], ot[:], axis=AX.X, op=OP.add)
                nc.scalar.activation(st[:], st[:], AF.Copy, scale=1.0 / DM)
                nc.vector.tensor_scalar(ot[:], ot[:], st[:, 0:1], None, OP.subtract)
                sq2 = st2.tile([128, DM], FP32, name="sq2", tag="h2sb")
                nc.vector.tensor_tensor(sq2[:], ot[:], ot[:], OP.mult)
                v2 = st2.tile([128, 1], FP32, name="v2", tag="v2")
                nc.vector.tensor_reduce(v2[:], sq2[:], axis=AX.X, op=OP.add)
                nc.scalar.activation(v2[:], v2[:], AF.Ln, bias=eps_ap[:], scale=1.0 / DM)
                nc.scalar.activation(v2[:], v2[:], AF.Exp, scale=-0.5)
                nc.vector.tensor_scalar(ot[:], ot[:], v2[:, 0:1], None, OP.mult)
                nc.vector.tensor_tensor(ot[:], ot[:], nfw_rep[:], OP.mult)
                nc.vector.tensor_tensor(ot[:], ot[:], nfb_rep[:], OP.add)
                nc.sync.dma_start(out_d[c * Q:(c + 1) * Q, :], ot[:])

    nc.compile()
    return nc


def _pack_fm(arr, pad_to=128):
    arr = np.asarray(arr)
    if arr.ndim == 1:
        arr = arr[:, None]
    F, W = arr.shape
    nblk = (F + pad_to - 1) // pad_to
    outp = np.zeros((pad_to, nblk * W), dtype=arr.dtype)
    for b_ in range(nblk):
        blk = arr[b_ * pad_to:(b_ + 1) * pad_to]
        outp[:blk.shape[0], b_ * W:(b_ + 1) * W] = blk
    return outp


def _prep_inputs(inputs):
    f32 = np.float32
    x = np.ascontiguousarray(np.asarray(inputs["x"], f32).reshape(T, DM))
    bprob = np.ascontiguousarray(np.asarray(inputs["boundary_prob"], f32).reshape(1, T))
    A = -np.exp(np.asarray(inputs["A_log"], f32))
    a_scales = A[0, 0, :]

    r = np.arange(128)
    # REPA[e', 128k + r] = (e' == 8k + r%8) for k<16 (contraction over e' in [0,128))
    repa = np.zeros((128, 16 * 128), f32)
    for k in range(16):
        repa[:, 128 * k:128 * (k + 1)] = (np.arange(128)[:, None] ==
                                          (8 * k + (r % 8))[None, :])
    repb = np.zeros((64, 8 * 128), f32)
    for kk in range(8):
        repb[:, 128 * kk:128 * (kk + 1)] = (np.arange(64)[:, None] ==
                                            (8 * kk + (r % 8))[None, :])
    # POOL[r, 128k + j] = (j == 8*(k%16) + r%8)
    poolm = np.zeros((128, NKT * 128), f32)
    for k in range(NKT):
        poolm[:, 128 * k:128 * (k + 1)] = ((8 * (k % 16) + (r % 8))[:, None] ==
                                           np.arange(128)[None, :])
    # REPN[n, r] = (n == r//8)   (Brep/Crep: B16 row n -> rows 8n..8n+7)
    repn = (np.arange(16)[:, None] == (r // 8)[None, :]).astype(f32)
    acol = a_scales[(r // 8) % 16].astype(f32)[:, None]

    maps = []
    for c in range(NC):
        sl = slice(c * ELOC, (c + 1) * ELOC)
        w_in = np.stack([_pack_fm(
            np.concatenate([np.asarray(inputs["in_proj_w"][i])[sl],
                            np.asarray(inputs["in_proj_w"][i])[E + c * ELOC:E + (c + 1) * ELOC]],
                           axis=0).T.astype(f32))
            for i in range(NL)])
        w_xp = np.stack([_pack_fm(np.asarray(inputs["x_proj_w"][i], f32)[:, sl].T)
                         for i in range(NL)])
        w_dt = np.stack([np.asarray(inputs["dt_proj_w"][i], f32)[sl].T
                         for i in range(NL)])
        dtb = np.stack([_pack_fm(np.asarray(inputs["dt_proj_b"][i], f32)[sl])
                        for i in range(NL)])
        w_out = np.stack([_pack_fm(np.asarray(inputs["out_proj_w"][i], f32)[:, sl].T)
                          for i in range(NL)])
        lnp = np.stack([_pack_fm(np.stack([np.asarray(inputs["ln_w"][i], f32),
                                           np.asarray(inputs["ln_b"][i], f32)], axis=1))
                        for i in range(NL)])
        gsl = slice(c * GDM, (c + 1) * GDM)
        cw1 = np.asarray(inputs["ctrl_w1"], f32)
        w_c1 = np.concatenate([_pack_fm(cw1[gsl, :DM].T),
                               _pack_fm(cw1[gsl, DM:DM + 1].T)], axis=1)
        w_c2 = np.concatenate([np.asarray(inputs["ctrl_w2"], f32)[:, gsl].T,
                               (np.asarray(inputs["ctrl_b2"], f32) / NC)[None, :]], axis=0)
        nrm = np.stack([np.asarray(inputs["normf_w"], f32), np.asarray(inputs["normf_b"], f32),
                        np.asarray(inputs["out_ln_w"], f32), np.asarray(inputs["out_ln_b"], f32)])
        nrmc = _pack_fm(np.stack([np.asarray(inputs["normf_w"], f32),
                                  np.asarray(inputs["normf_b"], f32)], axis=1))
        maps.append({
            "x": x, "bprob": bprob, "w_in": w_in,
            "conv_w": np.stack([_pack_fm(np.asarray(inputs["conv_w"][i], f32)[sl])
                                for i in range(NL)]),
            "conv_b": np.stack([_pack_fm(np.asarray(inputs["conv_b"][i], f32)[sl])
                                for i in range(NL)]),
            "w_xp": w_xp, "w_dt": w_dt, "dtb": dtb, "w_out": w_out, "lnp": lnp,
            "ssmd": np.stack([_pack_fm(np.asarray(inputs["ssm_D"][i], f32)[sl])
                              for i in range(NL)]),
            "w_c1": w_c1,
            "b_c1": np.asarray(inputs["ctrl_b1"], f32)[gsl][:, None],
            "w_c2": w_c2, "nrm": nrm, "nrmc": nrmc,
            "repa": repa, "repb": repb, "poolm": poolm, "repn": repn,
            "acol": acol,
        })
    return maps


def _cast_bf16(maps):
    import ml_dtypes
    for m in maps:
        for k in ("w_in", "w_xp", "w_dt", "w_out", "w_c1", "w_c2", "bprob",
                  "repa", "repb", "poolm", "repn"):
            m[k] = np.asarray(m[k], dtype=ml_dtypes.bfloat16)
    return maps


def kernel(**inputs):
    maps = _prep_inputs(inputs)
    A = -np.exp(np.asarray(inputs["A_log"], np.float32))
    a_scales = A[0, 0, :]
    for i in range(NL):
        assert np.allclose(A[i], np.broadcast_to(a_scales, (E, N)), rtol=1e-5, atol=1e-6), \
            "kernel assumes channel-independent A"
    if "nc" not in _CACHE:
        _CACHE["nc"] = _build()
    nc = _CACHE["nc"]
    _cast_bf16(maps)
    res = run_bass_kernel_spmd(nc, maps, list(range(NC)))
    kernel._res = res
    r0 = res.results[0]
    out = np.asarray(r0["out"], np.float32).reshape(B, L, DM)
    gate = np.asarray(r0["gate"], np.float32).reshape(B, L, DM)
    return out, gate



# revision 18
# speedup vs baseline: 1.7374x; 1.0550x over previous
"""Trainium2 Bass kernel for nn_BoundaryControlledMixer (4-layer Mamba stack +
boundary-controlled gate), tensor-parallel over d_inner across 8 NeuronCores.

Per core (owns E_loc = 192 of E = 1536 channels, full batch/sequence):
  - Activations flow feature-major [feat, token] so projections chain on the
    PE without transposes (matmul contracts the partition dim).
  - Selective scan: DVE tensor_tensor_scan (state = dA*state + b, fp32 state)
    over 24 row-tiles of the (n, e) grid: tile k holds rows r = n*8 + e_l,
    e = 8k + e_l, n = r//8, scanned along the full token axis [128, T].
    Batch reset via dA[:, L] = 0.  Inputs built by PE 0/1-replication
    (dt -> dtrep -> exp = dA) and log-doubling DMA (u = dt*xc -> ubrep),
    b = ubrep * Brep.  y = sum_n C*h via a 0/1 pooling matmul.
"""

import numpy as np

import concourse.bacc as bacc
import concourse.bass as bass
import concourse.mybir as mybir
import concourse.tile as tile
from concourse import masks
from concourse.bass_utils import run_bass_kernel_spmd

FP32 = mybir.dt.float32
BF16 = mybir.dt.bfloat16
AF = mybir.ActivationFunctionType
OP = mybir.AluOpType
AX = mybir.AxisListType

B, L, DM, NL = 2, 1024, 768, 4
E, N, K, R = 2 * DM, 16, 4, DM // 16
NC = 8
ELOC = E // NC            # 192
T = B * L                 # 2048
Q = 128
NCH = T // Q              # 16
EPS = 1e-5
DTILES = DM // 128        # 6
GDM = DM // NC            # 96
LPD = L + 2 * K           # padded per-batch xp row
NKT = N * ELOC // 128     # 24 scan tiles
RD = R + 1                # 49

_CACHE = {}
_DEBUG = False


def _etiles():
    return [(0, 128), (128, 64)]


def _build():
    nc = bacc.Bacc("TRN2", target_bir_lowering=False, debug=False)

    x_d = nc.dram_tensor("x", [T, DM], FP32, kind="ExternalInput")
    bprob_d = nc.dram_tensor("bprob", [1, T], BF16, kind="ExternalInput")
    w_in_d = nc.dram_tensor("w_in", [NL, 128, 6 * 2 * ELOC], BF16, kind="ExternalInput")
    conv_w_d = nc.dram_tensor("conv_w", [NL, 128, 2 * K], FP32, kind="ExternalInput")
    conv_b_d = nc.dram_tensor("conv_b", [NL, 128, 2], FP32, kind="ExternalInput")
    w_xp_d = nc.dram_tensor("w_xp", [NL, 128, 2 * (R + 2 * N)], BF16, kind="ExternalInput")
    w_dt_d = nc.dram_tensor("w_dt", [NL, R, ELOC], BF16, kind="ExternalInput")
    dtb_d = nc.dram_tensor("dtb", [NL, 128, 2], FP32, kind="ExternalInput")
    w_out_d = nc.dram_tensor("w_out", [NL, 128, 2 * DM], BF16, kind="ExternalInput")
    ln_d = nc.dram_tensor("lnp", [NL, 128, 12], FP32, kind="ExternalInput")
    ssmd_d = nc.dram_tensor("ssmd", [NL, 128, 2], FP32, kind="ExternalInput")
    w_c1_d = nc.dram_tensor("w_c1", [128, 7 * GDM], BF16, kind="ExternalInput")
    b_c1_d = nc.dram_tensor("b_c1", [GDM, 1], FP32, kind="ExternalInput")
    w_c2_d = nc.dram_tensor("w_c2", [GDM + 1, DM], BF16, kind="ExternalInput")
    nrm_d = nc.dram_tensor("nrm", [4, DM], FP32, kind="ExternalInput")
    nrmc_d = nc.dram_tensor("nrmc", [128, 12], FP32, kind="ExternalInput")
    repa_d = nc.dram_tensor("repa", [128, 16 * 128], BF16, kind="ExternalInput")
    repb_d = nc.dram_tensor("repb", [64, 8 * 128], BF16, kind="ExternalInput")
    pool_d = nc.dram_tensor("poolm", [128, NKT * 128], BF16, kind="ExternalInput")
    repn_d = nc.dram_tensor("repn", [16, 128], BF16, kind="ExternalInput")
    acol_d = nc.dram_tensor("acol", [128, 1], FP32, kind="ExternalInput")

    out_d = nc.dram_tensor("out", [T, DM], FP32, kind="ExternalOutput")
    gate_d = nc.dram_tensor("gate", [T, DM], FP32, kind="ExternalOutput")
    dbg = {}
    if _DEBUG:
        dbg["dt"] = nc.dram_tensor("dbg_dt", [ELOC, T], FP32, kind="ExternalOutput")
        dbg["yssm"] = nc.dram_tensor("dbg_yssm", [ELOC, T], FP32, kind="ExternalOutput")
        dbg["xc"] = nc.dram_tensor("dbg_xc", [ELOC, T], FP32, kind="ExternalOutput")
        dbg["hln"] = nc.dram_tensor("dbg_hln", [DM, T], FP32, kind="ExternalOutput")

    with tile.TileContext(nc) as tc:
        with tc.tile_pool(name="const", bufs=1) as constp, \
             tc.tile_pool(name="persist", bufs=1) as pers, \
             tc.tile_pool(name="wts", bufs=1) as wpool, \
             tc.tile_pool(name="act", bufs=1) as actp, \
             tc.tile_pool(name="st2", bufs=1) as st2, \
             tc.tile_pool(name="scn", bufs=1) as scn, \
             tc.tile_pool(name="ps_rep", bufs=1, space="PSUM") as ps_rep, \
             tc.tile_pool(name="ps_pool", bufs=1, space="PSUM") as ps_pool, \
             tc.tile_pool(name="ps_mm", bufs=2, space="PSUM") as ps_mm, \
             tc.tile_pool(name="dram", bufs=2, space="DRAM") as dramp:

            def pmm(shape, dt=FP32):
                return ps_mm.tile(shape, dt, name="pmm", tag="pmm")

            # ---------- constants ----------
            ident32 = constp.tile([128, 128], FP32)
            masks.make_identity(nc, ident32[:])
            ident16 = constp.tile([128, 128], BF16)
            masks.make_identity(nc, ident16[:])
            onesrow16 = constp.tile([1, 512], BF16)
            nc.gpsimd.memset(onesrow16[:], 1.0)
            halfcol32 = constp.tile([128, 1], FP32)
            nc.gpsimd.memset(halfcol32[:], 0.5)
            halfcol16 = constp.tile([128, 1], BF16)
            nc.gpsimd.memset(halfcol16[:], 0.5)
            eps_ap = constp.tile([128, 1], FP32)
            nc.gpsimd.memset(eps_ap[:], EPS)
            nrmc = constp.tile([128, 12], FP32)
            nc.sync.dma_start(nrmc[:], nrmc_d[:])
            repa = constp.tile([128, 16 * 128], BF16)
            nc.sync.dma_start(repa[:], repa_d[:])
            repb = constp.tile([64, 8 * 128], BF16)
            nc.sync.dma_start(repb[:], repb_d[:])
            poolm = constp.tile([128, NKT * 128], BF16)
            nc.sync.dma_start(poolm[:], pool_d[:])
            repn = constp.tile([16, 128], BF16)
            nc.sync.dma_start(repn[:], repn_d[:])
            acol = constp.tile([128, 1], FP32)
            nc.sync.dma_start(acol[:], acol_d[:])

            # ---------- x -> feature-major fp32 residual ----------
            residual = [pers.tile([128, T], FP32, name=f"res{j}") for j in range(DTILES)]
            for c in range(NCH):
                x_tm_c = st2.tile([128, DM], FP32, name="x_tm_c", tag="x_tm_c")
                nc.sync.dma_start(x_tm_c[:], x_d[c * Q:(c + 1) * Q, :])
                for j in range(DTILES):
                    pt = pmm([128, 128])
                    nc.tensor.transpose(pt[:], x_tm_c[:, j * 128:(j + 1) * 128], ident32[:])
                    nc.scalar.copy(residual[j][:, c * Q:(c + 1) * Q], pt[:])

            # ---------- fused feature-major LayerNorm ----------
            def ln_fm(lnw_aps, lnb_aps, consume, dbg_dst=None):
                stats16 = actp.tile([1, T], BF16, name="stats16", tag="stats16")
                stats16b = actp.tile([1, T], BF16, name="stats16b", tag="stats16b")

                for f in range(T // 512):
                    fs = slice(f * 512, (f + 1) * 512)
                    sp1 = pmm([1, 512])
                    sp2 = pmm([1, 512])
                    for j in range(DTILES):
                        nc.tensor.matmul(sp1[:], halfcol32, residual[j][:, fs],
                                         start=(j == 0), stop=(j == DTILES - 1))
                    nc.scalar.activation(stats16[0:1, fs], sp1[:], AF.Copy, scale=2.0 / DM)
                    for j in range(DTILES):
                        sqj = st2.tile([128, 512], BF16, name="ln_sqj", tag="ln_sqj")
                        nc.vector.tensor_tensor(sqj[:], residual[j][:, fs],
                                                residual[j][:, fs], OP.mult)
                        nc.tensor.matmul(sp2[:], halfcol16[:], sqj[:],
                                         start=(j == 0), stop=(j == DTILES - 1))
                    nc.scalar.activation(stats16b[0:1, fs], sp2[:], AF.Copy, scale=2.0 / DM)

                for f in range(T // 512):
                    fs = slice(f * 512, (f + 1) * 512)
                    rp = pmm([128, 512])
                    nc.tensor.matmul(rp[:], onesrow16[:1, :128], stats16[0:1, fs],
                                     start=True, stop=True)
                    meanr = st2.tile([128, 512], FP32, name="ln_meanr", tag="ln_meanr")
                    nc.scalar.copy(meanr[:], rp[:])
                    rp2 = pmm([128, 512])
                    nc.tensor.matmul(rp2[:], onesrow16[:1, :128], stats16b[0:1, fs],
                                     start=True, stop=True)
                    invr = st2.tile([128, 512], FP32, name="ln_invr", tag="ln_invr")
                    nc.scalar.copy(invr[:], rp2[:])
                    # var = E[x^2] - mean^2 ; inv = exp(-0.5*ln(var+eps))
                    c2r = st2.tile([128, 512], FP32, name="ln_c2r", tag="ln_c2r")
                    nc.vector.tensor_tensor(c2r[:], meanr[:], meanr[:], OP.mult)
                    nc.vector.tensor_tensor(invr[:], invr[:], c2r[:], OP.subtract)
                    nc.scalar.activation(invr[:], invr[:], AF.Ln, bias=eps_ap[:])
                    nc.scalar.activation(invr[:], invr[:], AF.Exp, scale=-0.5)
                    nc.vector.tensor_tensor(c2r[:], meanr[:], invr[:], OP.mult)
                    slices = []
                    for j in range(DTILES):
                        tmp = st2.tile([128, 512], BF16, name="ln_tmp", tag="ln_tmp", bufs=2)
                        nc.vector.tensor_tensor(tmp[:], residual[j][:, fs], invr[:], OP.mult)
                        nc.vector.tensor_tensor(tmp[:], tmp[:], c2r[:], OP.subtract)
                        hlnf = st2.tile([128, 512], BF16, name="hlnf", tag=f"hlnf{j}")
                        nc.scalar.activation(hlnf[:], tmp[:], AF.Identity,
                                             scale=lnw_aps[j], bias=lnb_aps[j])
                        slices.append(hlnf)
                        if dbg_dst is not None:
                            hld = st2.tile([128, 512], FP32, name="hld", tag="hld")
                            nc.vector.tensor_copy(hld[:], hlnf[:])
                            nc.sync.dma_start(dbg_dst[j * 128:(j + 1) * 128, fs], hld[:])
                    consume(f, slices)

            # ================= layers =================
            for li in range(NL):
                w_in = wpool.tile([128, 6 * 2 * ELOC], BF16, name="w_in_sb", tag="w_in_sb")
                nc.sync.dma_start(w_in[:], w_in_d[li])
                w_cw = wpool.tile([128, 2 * K], FP32, name="w_cw_sb", tag="w_cw_sb")
                nc.sync.dma_start(w_cw[:], conv_w_d[li])
                w_cb = wpool.tile([128, 2], FP32, name="w_cb_sb", tag="w_cb_sb")
                nc.sync.dma_start(w_cb[:], conv_b_d[li])
                w_xp = wpool.tile([128, 2 * (R + 2 * N)], BF16, name="w_xp_sb", tag="w_xp_sb")
                nc.sync.dma_start(w_xp[:], w_xp_d[li])
                w_dt = wpool.tile([R, ELOC], BF16, name="w_dt_sb", tag="w_dt_sb")
                nc.sync.dma_start(w_dt[:], w_dt_d[li])
                dtb = wpool.tile([128, 2], FP32, name="dtb_sb", tag="dtb_sb")
                nc.sync.dma_start(dtb[:], dtb_d[li])
                w_out = wpool.tile([128, 2 * DM], BF16, name="w_out_sb", tag="w_out_sb")
                nc.sync.dma_start(w_out[:], w_out_d[li])
                w_ln = wpool.tile([128, 12], FP32, name="w_ln_sb", tag="w_ln_sb")
                nc.sync.dma_start(w_ln[:], ln_d[li])
                w_D = wpool.tile([128, 2], FP32, name="w_D_sb", tag="w_D_sb")
                nc.sync.dma_start(w_D[:], ssmd_d[li])

                # ---- LN fused with in_proj ----
                xp_t = [actp.tile([128, B * LPD], BF16, name="xp_pad0", tag="xp_pad0"),
                        actp.tile([64, B * LPD], BF16, name="xp_pad1", tag="xp_pad1")]
                z_sb = [actp.tile([128, T], BF16, name="z_sb0", tag="z_sb0"),
                        actp.tile([64, T], BF16, name="z_sb1", tag="z_sb1")]
                for ti in range(2):
                    nc.vector.memset(xp_t[ti][:, 0:K], 0.0)
                    nc.vector.memset(xp_t[ti][:, LPD:LPD + K], 0.0)

                def padcol(fs, fl):
                    b_ = fs // L
                    off = b_ * LPD + K + (fs - b_ * L)
                    return slice(off, off + fl)

                def consume_inproj(f, sl6):
                    fs = f * 512
                    for mt in range(3):
                        pt = pmm([128, 512])
                        for kt in range(DTILES):
                            nc.tensor.matmul(
                                pt[:], w_in[:, kt * 384 + mt * 128:kt * 384 + (mt + 1) * 128],
                                sl6[kt][:], start=(kt == 0), stop=(kt == DTILES - 1))
                        if mt == 0:
                            nc.scalar.copy(xp_t[0][:, padcol(fs, 512)], pt[:])
                        elif mt == 1:
                            nc.scalar.copy(xp_t[1][:, padcol(fs, 512)], pt[0:64, :])
                            nc.scalar.copy(z_sb[0][0:64, fs:fs + 512], pt[64:128, :])
                        else:
                            nc.scalar.copy(z_sb[0][64:128, fs:fs + 512], pt[0:64, :])
                            nc.scalar.copy(z_sb[1][0:64, fs:fs + 512], pt[64:128, :])

                ln_fm([w_ln[:, 2 * j:2 * j + 1] for j in range(DTILES)],
                      [w_ln[:, 2 * j + 1:2 * j + 2] for j in range(DTILES)],
                      consume_inproj,
                      dbg_dst=dbg["hln"] if (_DEBUG and li == 0) else None)

                # ---- conv + silu ----
                xc = [actp.tile([128, T], BF16, name="xc0", tag="xc0"),
                      actp.tile([64, T], BF16, name="xc1", tag="xc1")]
                for ti, (eo, el) in enumerate(_etiles()):
                    for b_ in range(B):
                        acc = st2.tile([el, L], FP32, name="cacc", tag="cacc", bufs=2)
                        cb = b_ * LPD + K
                        nc.vector.tensor_scalar(acc[:], xp_t[ti][:el, cb - 3:cb - 3 + L],
                                                w_cw[0:el, ti * K:ti * K + 1], None, OP.mult)
                        for j in range(1, K):
                            nc.vector.scalar_tensor_tensor(
                                acc[:], xp_t[ti][:el, cb - 3 + j:cb - 3 + j + L],
                                w_cw[0:el, ti * K + j:ti * K + j + 1],
                                acc[:], OP.mult, OP.add)
                        nc.scalar.activation(xc[ti][:el, b_ * L:(b_ + 1) * L], acc[:],
                                             AF.Silu, bias=w_cb[0:el, ti:ti + 1])
                if _DEBUG and li == 0:
                    for ti, (eo, el) in enumerate(_etiles()):
                        xcd = st2.tile([el, T], FP32, name="xcd", tag="xcd")
                        nc.vector.tensor_copy(xcd[:], xc[ti][:el, :])
                        nc.sync.dma_start(dbg["xc"][eo:eo + el, :], xcd[:])

                # ---- x_proj partial + AllReduce (bf16 wire, Shared out) ----
                dbl_in = dramp.tile([R + 2 * N, T], BF16, name="dbl_in", tag="dbl_in")
                dbl_out = dramp.tile([R + 2 * N, T], BF16, name="dbl_out",
                                     tag="dbl_out", addr_space="Shared", bufs=NL)
                for f in range(T // 512):
                    fs = slice(f * 512, (f + 1) * 512)
                    pt = pmm([80, 512])
                    for ti, (eo, el) in enumerate(_etiles()):
                        nc.tensor.matmul(pt[:], w_xp[0:el, ti * 80:(ti + 1) * 80],
                                         xc[ti][:el, fs], start=(ti == 0), stop=(ti == 1))
                    dblf = st2.tile([80, 512], BF16, name="dblf", tag="dblf")
                    nc.scalar.copy(dblf[:], pt[:])
                    nc.sync.dma_start(dbl_in[:, fs], dblf[:])
                nc.gpsimd.collective_compute("AllReduce", OP.add,
                                             replica_groups=[list(range(NC))],
                                             ins=[dbl_in[:]], outs=[dbl_out[:]])

                # ---- dt path: softplus(w_dt @ dbl + b) e-major ----
                dtf16 = scn.tile([R, T], BF16, name="dtf16", tag="dtf16", bufs=2)
                nc.sync.dma_start(dtf16[:], dbl_out[0:R, :])

                dt16 = [scn.tile([128, T], BF16, name="dt16_0", tag="dt16_0"),
                        scn.tile([64, T], BF16, name="dt16_1", tag="dt16_1")]
                for ti, (eo, el) in enumerate(_etiles()):
                    for h_ in range(4):
                        csl = slice(h_ * 512, (h_ + 1) * 512)
                        ptd = ps_rep.tile([128, 512], FP32, name="ptd", tag="rep",
                                          bufs=4)
                        nc.tensor.matmul(ptd[0:el, :],
                                         w_dt[:, eo:eo + el], dtf16[:, csl],
                                         start=True, stop=True)
                        e1 = scn.tile([el, 512], FP32, name="e1", tag="dstage")
                        nc.scalar.activation(e1[:], ptd[0:el, :], AF.Exp,
                                             bias=dtb[0:el, ti:ti + 1])
                        nc.scalar.activation(dt16[ti][:el, csl], e1[:], AF.Ln,
                                             bias=1.0)
                if _DEBUG and li == 0:
                    dtd = st2.tile([128, T], FP32, name="dtd", tag="dtd")
                    nc.vector.tensor_copy(dtd[:], dt16[0][:])
                    nc.sync.dma_start(dbg["dt"][0:128, :], dtd[:])
                    dtd2 = st2.tile([64, T], FP32, name="dtd2", tag="dtd")
                    nc.vector.tensor_copy(dtd2[:], dt16[1][:])
                    nc.sync.dma_start(dbg["dt"][128:192, :], dtd2[:])

                # ---- u = dt * xc ; B/C replication ----
                u16 = [scn.tile([128, T], BF16, name="u16_0", tag="u16_0"),
                       scn.tile([64, T], BF16, name="u16_1", tag="u16_1")]
                for ti, (eo, el) in enumerate(_etiles()):
                    nc.vector.tensor_tensor(u16[ti][:el, :], dt16[ti][:el, :],
                                            xc[ti][:el, :], OP.mult)

                Brep = scn.tile([128, T], BF16, name="Brep", tag="Brep")
                Crep = scn.tile([128, T], BF16, name="Crep", tag="Crep")
                for si, dst in ((0, Brep), (1, Crep)):
                    src16 = scn.tile([16, T], BF16, name=f"bc16_{si}", tag="dtf16", bufs=2)
                    nc.sync.dma_start(src16[:], dbl_out[R + si * N:R + (si + 1) * N, :])
                    for h_ in range(4):
                        csl = slice(h_ * 512, (h_ + 1) * 512)
                        prr = ps_rep.tile([128, 512], FP32, name="prr", tag="rep",
                                          bufs=4)
                        nc.tensor.matmul(prr[:], repn[:], src16[:, csl],
                                         start=True, stop=True)
                        nc.scalar.copy(dst[:, csl], prr[:])

                # ---- scan tiles ----
                # Per (group, batch): pool accumulator [128, L] (2 PSUM banks);
                # dt/u replication via PE matmul into [128, L] PSUM (2 banks,
                # bufs=2), consumed by scalar (exp -> dA, copy -> urep).
                y_fm = [actp.tile([128, T], BF16, name="yfm0", tag="xp_pad0"),
                        actp.tile([64, T], BF16, name="yfm1", tag="xp_pad1")]
                for g, (ks, el_in) in enumerate(((range(0, 16), 128),
                                                 (range(16, 24), 64))):
                    rows_out = 128 if g == 0 else 64
                    repm = repa if g == 0 else repb
                    for b_ in range(B):
                        bsl = slice(b_ * L, (b_ + 1) * L)
                        pp = ps_pool.tile([128, L], FP32, name="pp", tag="pp")
                        for k in ks:
                            kk = k - 16 * g
                            # u replication via PE first (feeds bk)
                            ubrep = scn.tile([128, L], BF16, name="ubrep",
                                             tag="ubrep", bufs=2)
                            for cc in range(2):
                                psl = slice(cc * 512, (cc + 1) * 512)
                                csl = slice(b_ * L + cc * 512, b_ * L + (cc + 1) * 512)
                                pru = ps_rep.tile([128, 512], FP32, name="pru",
                                                  tag="rep", bufs=4)
                                nc.tensor.matmul(pru[:],
                                                 repm[:, kk * 128:(kk + 1) * 128],
                                                 u16[g][:el_in, csl],
                                                 start=True, stop=True)
                                nc.scalar.copy(ubrep[:, psl], pru[:])
                            # dt replication -> dA = exp(a * dtrep)
                            dA = scn.tile([128, L], FP32, name="dA", tag="dA", bufs=2)
                            for cc in range(2):
                                psl = slice(cc * 512, (cc + 1) * 512)
                                csl = slice(b_ * L + cc * 512, b_ * L + (cc + 1) * 512)
                                prp = ps_rep.tile([128, 512], FP32, name="prp",
                                                  tag="rep", bufs=4)
                                nc.tensor.matmul(prp[:],
                                                 repm[:, kk * 128:(kk + 1) * 128],
                                                 dt16[g][:el_in, csl],
                                                 start=True, stop=True)
                                nc.scalar.activation(dA[:, psl], prp[:], AF.Exp,
                                                     scale=acol[:, 0:1])
                            bk = scn.tile([128, L], BF16, name="bk", tag="bk", bufs=2)
                            nc.vector.tensor_tensor(bk[:], ubrep[:], Brep[:, bsl],
                                                    OP.mult)
                            hk = scn.tile([128, L], BF16, name="hk", tag="dtf16", bufs=2)
                            nc.vector.tensor_tensor_scan(hk[:], dA[:], bk[:], 0.0,
                                                         OP.mult, OP.add)
                            yck = scn.tile([128, L], BF16, name="yck", tag="ubrep",
                                           bufs=2)
                            nc.vector.tensor_tensor(yck[:], hk[:], Crep[:, bsl],
                                                    OP.mult)
                            # pool: accumulate sum_n C*h into e-rows
                            for cc in range(2):
                                psl = slice(cc * 512, (cc + 1) * 512)
                                nc.tensor.matmul(pp[0:rows_out, psl],
                                                 poolm[:, k * 128:k * 128 + rows_out],
                                                 yck[:, cc * 512:(cc + 1) * 512],
                                                 start=(k == ks[0]), stop=(k == ks[-1]))
                        nc.scalar.copy(y_fm[g][0:rows_out, bsl], pp[0:rows_out, :])
                if _DEBUG and li == 0:
                    for ti, (eo, el) in enumerate(_etiles()):
                        ydd = st2.tile([el, T], FP32, name="ydd", tag="xcd")
                        nc.vector.tensor_copy(ydd[:], y_fm[ti][:el, :])
                        nc.sync.dma_start(dbg["yssm"][eo:eo + el, :], ydd[:])

                # ---- D-term, z-gate ----
                for ti, (eo, el) in enumerate(_etiles()):
                    nc.vector.scalar_tensor_tensor(y_fm[ti][:el, :], xc[ti][:el, :],
                                                   w_D[0:el, ti:ti + 1], y_fm[ti][:el, :],
                                                   OP.mult, OP.add)
                    nc.scalar.activation(z_sb[ti][:el, :], z_sb[ti][:el, :], AF.Silu)
                    nc.vector.tensor_tensor(y_fm[ti][:el, :], y_fm[ti][:el, :],
                                            z_sb[ti][:el, :], OP.mult)

                # ---- out_proj partial + AllReduce (split halves, overlap) ----
                op_outs = []
                for b_ in range(B):
                    op_in = dramp.tile([DM, L], BF16, name="op_in", tag="op_in")
                    op_out = dramp.tile([DM, L], BF16, name="op_out", tag="op_out",
                                        addr_space="Shared", bufs=2 * NL)
                    op_outs.append(op_out)
                    for mt in range(DTILES):
                        for f in range(2):
                            fs = slice(b_ * L + f * 512, b_ * L + (f + 1) * 512)
                            pt = pmm([128, 512])
                            for ti, (eo, el) in enumerate(_etiles()):
                                nc.tensor.matmul(
                                    pt[:],
                                    w_out[0:el, ti * DM + mt * 128:ti * DM + (mt + 1) * 128],
                                    y_fm[ti][:el, fs], start=(ti == 0), stop=(ti == 1))
                            opf = st2.tile([128, 512], BF16, name="opf", tag="opf",
                                           bufs=2)
                            nc.scalar.copy(opf[:], pt[:])
                            nc.sync.dma_start(
                                op_in[mt * 128:(mt + 1) * 128,
                                      f * 512:(f + 1) * 512], opf[:])
                    nc.gpsimd.collective_compute("AllReduce", OP.add,
                                                 replica_groups=[list(range(NC))],
                                                 ins=[op_in[:]], outs=[op_out[:]])
                for j in range(DTILES):
                    for b_ in range(B):
                        hs_f = st2.tile([128, L], BF16, name="hs_f", tag="hs_f",
                                        bufs=2)
                        nc.sync.dma_start(hs_f[:],
                                          op_outs[b_][j * 128:(j + 1) * 128, :])
                        nc.vector.tensor_tensor(
                            residual[j][:, b_ * L:(b_ + 1) * L],
                            residual[j][:, b_ * L:(b_ + 1) * L], hs_f[:], OP.add)

            # ================= final stage =================
            mixed = [actp.tile([128, T], BF16, name=f"mx{j}", tag=t)
                     for j, t in enumerate(
                         ["xp_pad0", "xp_pad1", "xc0", "xc1", "z_sb0", "z_sb1"])]

            def consume_mixed(f, sl6):
                fs = slice(f * 512, (f + 1) * 512)
                for j in range(DTILES):
                    nc.vector.tensor_copy(mixed[j][:, fs], sl6[j][:])

            ln_fm([nrmc[:, 2 * j:2 * j + 1] for j in range(DTILES)],
                  [nrmc[:, 2 * j + 1:2 * j + 2] for j in range(DTILES)],
                  consume_mixed)

            brow = actp.tile([1, T], BF16, name="brow", tag="stats16")
            nc.sync.dma_start(brow[:], bprob_d[:])

            wc1 = wpool.tile([128, 7 * GDM], BF16, name="wc1", tag="w_in_sb")
            nc.sync.dma_start(wc1[:], w_c1_d[:])
            bc1 = wpool.tile([GDM, 1], FP32, name="bc1", tag="w_cb_sb")
            nc.sync.dma_start(bc1[:], b_c1_d[:])
            wc2 = wpool.tile([GDM + 1, DM], BF16, name="wc2", tag="w_out_sb")
            nc.sync.dma_start(wc2[:], w_c2_d[:])

            h1 = actp.tile([GDM + 1, T], BF16, name="h1", tag="stats16b")
            nc.vector.memset(h1[GDM:GDM + 1, :], 1.0)
            for f in range(T // 512):
                fs = slice(f * 512, (f + 1) * 512)
                xfb = [st2.tile([128, 512], BF16, name=f"xfb{j}", tag=f"hlnf{j}")
                       for j in range(DTILES)]
                for c4 in range(4):
                    c = f * 4 + c4
                    x_tm_c = st2.tile([128, DM], FP32, name="x_tm_c3", tag="x_tm_c")
                    nc.sync.dma_start(x_tm_c[:], x_d[c * Q:(c + 1) * Q, :])
                    for j in range(DTILES):
                        ptt = pmm([128, 128])
                        nc.tensor.transpose(ptt[:], x_tm_c[:, j * 128:(j + 1) * 128],
                                            ident32[:])
                        nc.scalar.copy(xfb[j][:, c4 * 128:(c4 + 1) * 128], ptt[:])
                pt = pmm([GDM, 512])
                for kt in range(DTILES):
                    nc.tensor.matmul(pt[:], wc1[:, kt * GDM:(kt + 1) * GDM],
                                     xfb[kt][:], start=(kt == 0), stop=False)
                nc.tensor.matmul(pt[:], wc1[0:1, 6 * GDM:7 * GDM], brow[:, fs],
                                 start=False, stop=True)
                nc.scalar.activation(h1[0:GDM, fs], pt[:], AF.Silu, bias=bc1[:, 0:1])

            g_in = dramp.tile([T, DM], BF16, name="g_in", tag="g_in")
            g_out = dramp.tile([T, DM], BF16, name="g_out", tag="g_out",
                               addr_space="Shared")
            for c in range(NCH):
                h2sb = st2.tile([128, DM], BF16, name="h2sb", tag="h2sb")
                for fs2 in range(2):
                    pt = pmm([128, 384])
                    nc.tensor.matmul(pt[:], h1[:, c * Q:(c + 1) * Q],
                                     wc2[:, fs2 * 384:(fs2 + 1) * 384],
                                     start=True, stop=True)
                    nc.scalar.copy(h2sb[:, fs2 * 384:(fs2 + 1) * 384], pt[:])
                nc.sync.dma_start(g_in[c * Q:(c + 1) * Q, :], h2sb[:])
            nc.gpsimd.collective_compute("AllReduce", OP.add,
                                         replica_groups=[list(range(NC))],
                                         ins=[g_in[:]], outs=[g_out[:]])

            n16 = actp.tile([1, DM], BF16, name="n16", tag="n16")
            n16b = actp.tile([1, DM], BF16, name="n16b", tag="n16b")
            nr32 = st2.tile([1, DM], FP32, name="nr32", tag="h2sb")
            nc.sync.dma_start(nr32[:], nrm_d[2:3, :])
            nc.vector.tensor_copy(n16[:], nr32[:])
            nr32b = st2.tile([1, DM], FP32, name="nr32b", tag="h2sb")
            nc.sync.dma_start(nr32b[:], nrm_d[3:4, :])
            nc.vector.tensor_copy(n16b[:], nr32b[:])
            nfw_rep = actp.tile([128, DM], BF16, name="nfw_rep", tag="nfw_rep")
            nfb_rep = actp.tile([128, DM], BF16, name="nfb_rep", tag="nfb_rep")
            for fs2 in range(2):
                rp = pmm([128, 384])
                nc.tensor.matmul(rp[:], onesrow16[:1, :128],
                                 n16[0:1, fs2 * 384:(fs2 + 1) * 384], start=True, stop=True)
                nc.scalar.copy(nfw_rep[:, fs2 * 384:(fs2 + 1) * 384], rp[:])
                rp2 = pmm([128, 384])
                nc.tensor.matmul(rp2[:], onesrow16[:1, :128],
                                 n16b[0:1, fs2 * 384:(fs2 + 1) * 384], start=True, stop=True)
                nc.scalar.copy(nfb_rep[:, fs2 * 384:(fs2 + 1) * 384], rp2[:])

            for c in range(NCH):
                mixed_tm = st2.tile([128, DM], BF16, name="mixed_tm", tag="mixed_tm")
                for j in range(DTILES):
                    ptt = pmm([128, 128], BF16)
                    nc.tensor.transpose(ptt[:], mixed[j][:, c * Q:(c + 1) * Q], ident16[:])
                    nc.scalar.copy(mixed_tm[:, j * 128:(j + 1) * 128], ptt[:])
                xt = st2.tile([128, DM], FP32, name="xt", tag="x_tm_c")
                nc.sync.dma_start(xt[:], x_d[c * Q:(c + 1) * Q, :])
                gt16 = st2.tile([128, DM], BF16, name="gt16", tag="gt16", bufs=2)
                nc.sync.dma_start(gt16[:], g_out[c * Q:(c + 1) * Q, :])
                gt = st2.tile([128, DM], FP32, name="gt", tag="cacc", bufs=2)
                nc.scalar.activation(gt[:], gt16[:], AF.Sigmoid)
                nc.sync.dma_start(gate_d[c * Q:(c + 1) * Q, :], gt[:])
                ot = st2.tile([128, DM], FP32, name="ot", tag="ot", bufs=1)
                nc.vector.tensor_tensor(ot[:], mixed_tm[:], xt[:], OP.subtract)
                nc.vector.tensor_tensor(ot[:], ot[:], gt[:], OP.mult)
                nc.vector.tensor_tensor(ot[:], ot[:], xt[:], OP.add)
                st = st2.tile([128, 1], FP32, name="st", tag="st")
                nc.vector.tensor_reduce(st[:], ot[:], axis=AX.X, op=OP.add)
                nc.scalar.activation(st[:], st[:], AF.Copy, scale=1.0 / DM)
                nc.vector.tensor_scalar(ot[:], ot[:], st[:, 0:1], None, OP.subtract)
                sq2 = st2.tile([128, DM], FP32, name="sq2", tag="h2sb")
                nc.vector.tensor_tensor(sq2[:], ot[:], ot[:], OP.mult)
                v2 = st2.tile([128, 1], FP32, name="v2", tag="v2")
                nc.vector.tensor_reduce(v2[:], sq2[:], axis=AX.X, op=OP.add)
                nc.scalar.activation(v2[:], v2[:], AF.Ln, bias=eps_ap[:], scale=1.0 / DM)
                nc.scalar.activation(v2[:], v2[:], AF.Exp, scale=-0.5)
                nc.vector.tensor_scalar(ot[:], ot[:], v2[:, 0:1], None, OP.mult)
                nc.vector.tensor_tensor(ot[:], ot[:], nfw_rep[:], OP.mult)
                nc.vector.tensor_tensor(ot[:], ot[:], nfb_rep[:], OP.add)
                nc.sync.dma_start(out_d[c * Q:(c + 1) * Q, :], ot[:])

    nc.compile()
    return nc


def _pack_fm(arr, pad_to=128):
    arr = np.asarray(arr)
    if arr.ndim == 1:
        arr = arr[:, None]
    F, W = arr.shape
    nblk = (F + pad_to - 1) // pad_to
    outp = np.zeros((pad_to, nblk * W), dtype=arr.dtype)
    for b_ in range(nblk):
        blk = arr[b_ * pad_to:(b_ + 1) * pad_to]
        outp[:blk.shape[0], b_ * W:(b_ + 1) * W] = blk
    return outp


def _prep_inputs(inputs):
    f32 = np.float32
    x = np.ascontiguousarray(np.asarray(inputs["x"], f32).reshape(T, DM))
    bprob = np.ascontiguousarray(np.asarray(inputs["boundary_prob"], f32).reshape(1, T))
    A = -np.exp(np.asarray(inputs["A_log"], f32))
    a_scales = A[0, 0, :]

    r = np.arange(128)
    # REPA[e', 128k + r] = (e' == 8k + r%8) for k<16 (contraction over e' in [0,128))
    repa = np.zeros((128, 16 * 128), f32)
    for k in range(16):
        repa[:, 128 * k:128 * (k + 1)] = (np.arange(128)[:, None] ==
                                          (8 * k + (r % 8))[None, :])
    repb = np.zeros((64, 8 * 128), f32)
    for kk in range(8):
        repb[:, 128 * kk:128 * (kk + 1)] = (np.arange(64)[:, None] ==
                                            (8 * kk + (r % 8))[None, :])
    # POOL[r, 128k + j] = (j == 8*(k%16) + r%8)
    poolm = np.zeros((128, NKT * 128), f32)
    for k in range(NKT):
        poolm[:, 128 * k:128 * (k + 1)] = ((8 * (k % 16) + (r % 8))[:, None] ==
                                           np.arange(128)[None, :])
    # REPN[n, r] = (n == r//8)   (Brep/Crep: B16 row n -> rows 8n..8n+7)
    repn = (np.arange(16)[:, None] == (r // 8)[None, :]).astype(f32)
    acol = a_scales[(r // 8) % 16].astype(f32)[:, None]

    maps = []
    for c in range(NC):
        sl = slice(c * ELOC, (c + 1) * ELOC)
        w_in = np.stack([_pack_fm(
            np.concatenate([np.asarray(inputs["in_proj_w"][i])[sl],
                            np.asarray(inputs["in_proj_w"][i])[E + c * ELOC:E + (c + 1) * ELOC]],
                           axis=0).T.astype(f32))
            for i in range(NL)])
        w_xp = np.stack([_pack_fm(np.asarray(inputs["x_proj_w"][i], f32)[:, sl].T)
                         for i in range(NL)])
        w_dt = np.stack([np.asarray(inputs["dt_proj_w"][i], f32)[sl].T
                         for i in range(NL)])
        dtb = np.stack([_pack_fm(np.asarray(inputs["dt_proj_b"][i], f32)[sl])
                        for i in range(NL)])
        w_out = np.stack([_pack_fm(np.asarray(inputs["out_proj_w"][i], f32)[:, sl].T)
                          for i in range(NL)])
        lnp = np.stack([_pack_fm(np.stack([np.asarray(inputs["ln_w"][i], f32),
                                           np.asarray(inputs["ln_b"][i], f32)], axis=1))
                        for i in range(NL)])
        gsl = slice(c * GDM, (c + 1) * GDM)
        cw1 = np.asarray(inputs["ctrl_w1"], f32)
        w_c1 = np.concatenate([_pack_fm(cw1[gsl, :DM].T),
                               _pack_fm(cw1[gsl, DM:DM + 1].T)], axis=1)
        w_c2 = np.concatenate([np.asarray(inputs["ctrl_w2"], f32)[:, gsl].T,
                               (np.asarray(inputs["ctrl_b2"], f32) / NC)[None, :]], axis=0)
        nrm = np.stack([np.asarray(inputs["normf_w"], f32), np.asarray(inputs["normf_b"], f32),
                        np.asarray(inputs["out_ln_w"], f32), np.asarray(inputs["out_ln_b"], f32)])
        nrmc = _pack_fm(np.stack([np.asarray(inputs["normf_w"], f32),
                                  np.asarray(inputs["normf_b"], f32)], axis=1))
        maps.append({
            "x": x, "bprob": bprob, "w_in": w_in,
            "conv_w": np.stack([_pack_fm(np.asarray(inputs["conv_w"][i], f32)[sl])
                                for i in range(NL)]),
            "conv_b": np.stack([_pack_fm(np.asarray(inputs["conv_b"][i], f32)[sl])
                                for i in range(NL)]),
            "w_xp": w_xp, "w_dt": w_dt, "dtb": dtb, "w_out": w_out, "lnp": lnp,
            "ssmd": np.stack([_pack_fm(np.asarray(inputs["ssm_D"][i], f32)[sl])
                              for i in range(NL)]),
            "w_c1": w_c1,
            "b_c1": np.asarray(inputs["ctrl_b1"], f32)[gsl][:, None],
            "w_c2": w_c2, "nrm": nrm, "nrmc": nrmc,
            "repa": repa, "repb": repb, "poolm": poolm, "repn": repn,
            "acol": acol,
        })
    return maps


def _cast_bf16(maps):
    import ml_dtypes
    for m in maps:
        for k in ("w_in", "w_xp", "w_dt", "w_out", "w_c1", "w_c2", "bprob",
                  "repa", "repb", "poolm", "repn"):
            m[k] = np.asarray(m[k], dtype=ml_dtypes.bfloat16)
    return maps


def kernel(**inputs):
    maps = _prep_inputs(inputs)
    A = -np.exp(np.asarray(inputs["A_log"], np.float32))
    a_scales = A[0, 0, :]
    for i in range(NL):
        assert np.allclose(A[i], np.broadcast_to(a_scales, (E, N)), rtol=1e-5, atol=1e-6), \
            "kernel assumes channel-independent A"
    if "nc" not in _CACHE:
        _CACHE["nc"] = _build()
    nc = _CACHE["nc"]
    _cast_bf16(maps)
    res = run_bass_kernel_spmd(nc, maps, list(range(NC)))
    kernel._res = res
    r0 = res.results[0]
    out = np.asarray(r0["out"], np.float32).reshape(B, L, DM)
    gate = np.asarray(r0["gate"], np.float32).reshape(B, L, DM)
    return out, gate

